# revision 2
# baseline (speedup 1.0000x reference)
"""EnhancedChannelFilter Trainium2 kernel (pair-interleaved layout).

Full inputs in, full outputs out. Pure data-parallel over 8 NeuronCores
(4 images each). Channels are pair-interleaved on SBUF: channel c = 2p+u
lives at partition p, sub-row u, so tiles are [128, 2, pixels].

Per core, per image:
  1. x is shipped from the host as bf16 [128, 2, HW] (layout/precision prep;
     end-to-end numerics validated at rel err 8.9e-3 vs the 2e-2 budget).
  2. Packet-loss mask shipped as fp8 [8, 2, HW] in channel-group space,
     expanded on the PE by a 0/1 DoubleRow matmul to [128, NT] per n-tile
     (mask is constant across channel pairs, so one expand serves both
     sub-rows).
  3. xm = x * mask fused with the SE row-sum via DVE STT accum_out (the
     f32 accumulation happens pre-bf16-rounding, keeping the SE path exact).
  4. SE chain (fc1 -> relu -> fc2 -> sigmoid -> +bias -> relu) on PE/ACT/DVE;
     1/HW and rate*adapt_w - threshold folded into host-packed weights.
  5. det GEMM in fp8e4 DoubleRow (4x MAC rate; xq fp8 copies produced on the
     otherwise-idle GPSIMD), rec1/rec2 in bf16 (1 col/cycle); sigmoid/relu
     PSUM evictions on ACT, zh = sigmoid(det)*xm on DVE, final per-channel
     scale split DVE/ACT, bf16 output tiles with paired out DMAs (host
     upcasts to f32).

Scheduling: a 2-deep software pipeline over flat (image, tile) windows --
window w emits det(w+1), rec1(w), rec2(w-1) plus one phase-1 tile of the
next image -- so the PE never waits on the det->sigmoid->zh chain and all
four engines run concurrently at ~90% of the PE's 2.5us/window.
"""

import math

import numpy as np
import ml_dtypes

B, C, H, W = 32, 256, 56, 56
HW = H * W              # 3136
NCORES = 8
BC = B // NCORES        # images per core
NT = 448                # pixels per n-tile
NTILES = HW // NT       # 7
EPC = 1472 // 4         # f32 elements per packet chunk (368)
QG = 16                 # channel-group size: gcd(EPC, C)
UPC = EPC // QG         # 23 channel-group-units per chunk

_CACHE: dict = {}


# ---------------------------------------------------------------------------
# Workaround: this walrus build enforces 1 sync wait per instruction (2 for
# EventSemaphore), but the Tile framework attaches several to its exit drain.
# Splitting extra waits onto dedicated same-engine NOPs placed immediately
# before the instruction is semantically identical.
# ---------------------------------------------------------------------------
def _split_multiwaits(nc, mybir):
    n = 0
    for bb in nc.m.functions[0].blocks:
        lst = bb.instructions
        for inst in list(lst):
            si = inst.sync_info
            if si is None or not si.on_wait:
                continue
            cap = 2 if isinstance(inst, mybir.InstEventSemaphore) else 1
            waits = list(si.on_wait)
            if len(waits) <= cap:
                continue
            eng = nc.engines[inst.engine]
            extra = []
            for wt in waits[:-cap]:
                nop = eng.nop(nofuse=True).ins
                nop.sync_info = mybir.SyncInfo(on_wait=[wt], on_update=[])
                nc.cur_bb.bb.instructions.remove(nop)
                extra.append(nop)
            si.on_wait = waits[-cap:]
            idx = lst.index(inst)
            lst[idx:idx] = extra
            n += 1
    return n


def _build(debug=False, repeat=0, det_fp8=True, xq_eng="pool", sched="pipe",
           out_bf16=True, sig2=True, mc_dve=True, xq_b0=None, unroll=1,
           mask_dma=False):
    import concourse.bass as bass
    import concourse.tile as tile
    import concourse.mybir as mybir

    f32 = mybir.dt.float32
    bf16 = mybir.dt.bfloat16
    fp8 = mybir.dt.float8e4
    DR = mybir.MatmulPerfMode.DoubleRow
    MULT = mybir.AluOpType.mult
    BYPASS = mybir.AluOpType.bypass
    SIGMOID = mybir.ActivationFunctionType.Sigmoid
    COPY = mybir.ActivationFunctionType.Copy
    RELU = mybir.ActivationFunctionType.Relu

    dp_bufs = (2 if mask_dma else 1) if not sig2 else 2
    nc = bass.Bass("TRN2", target_bir_lowering=False, debug=False)

    x_d = nc.dram_tensor("x", [BC, 128, 2, HW], bf16, kind="ExternalInput").ap()
    if mask_dma:
        m16_d = nc.dram_tensor("m16", [BC, 16, 1, HW], fp8,
                               kind="ExternalInput").ap()
    else:
        m16_d = nc.dram_tensor("m16", [BC, 8, 2, HW], fp8,
                               kind="ExternalInput").ap()
    # bf16 GEMM weights: [128, ncol, 128] with col blocks
    #   det (u,t) 4 cols (absent when det_fp8), rec1 (kt,u,t) 8, rec2 (u,h) 4
    nbcol = (0 if det_fp8 else 4) + 8 + 4
    wbf_d = nc.dram_tensor("wbf", [128, nbcol, 128], bf16, kind="ExternalInput").ap()
    if det_fp8:
        wf8_d = nc.dram_tensor("wf8", [128, 2, 2, 128], fp8, kind="ExternalInput").ap()
    if not mask_dma:
        esm_d = nc.dram_tensor("esm", [8, 2, 128], fp8,
                               kind="ExternalInput").ap()
    # f32 smalls: fc1 (u) 2x16 cols then abias (h) 2x1
    wfc_d = nc.dram_tensor("wfc", [128, 34], f32, kind="ExternalInput").ap()
    wsm_d = nc.dram_tensor("wsm", [16, 256], f32, kind="ExternalInput").ap()
    odt = bf16 if out_bf16 else f32
    out_d = nc.dram_tensor("out", [BC, 2, 128, HW], odt, kind="ExternalOutput").ap()
    if debug:
        dxm_d = nc.dram_tensor("dxm", [BC, 128, 2, HW], f32, kind="ExternalOutput").ap()
        dsg_d = nc.dram_tensor("dsg", [BC, 128, 2, HW], f32, kind="ExternalOutput").ap()
        dmc_d = nc.dram_tensor("dmc", [BC, 2, 128, 1], f32, kind="ExternalOutput").ap()
        dy_d = nc.dram_tensor("dy", [BC, 128, 2, 8], f32, kind="ExternalOutput").ap()

    with tile.TileContext(nc) as tc:
        with (
            tc.tile_pool(name="consts", bufs=1) as cpool,
            tc.tile_pool(name="xin", bufs=2) as xpool,
            tc.tile_pool(name="xm", bufs=2) as xmpool,
            tc.tile_pool(name="xq", bufs=2) as xqpool,
            tc.tile_pool(name="m16", bufs=2) as m16pool,
            tc.tile_pool(name="sg", bufs=3) as sgpool,
            tc.tile_pool(name="zh", bufs=3) as zhpool,
            tc.tile_pool(name="r1", bufs=6) as r1pool,
            tc.tile_pool(name="osb", bufs=2) as opool,
            tc.tile_pool(name="ysum", bufs=2) as ypool,
            tc.tile_pool(name="mch", bufs=4) as mcpool,
            tc.tile_pool(name="sesb", bufs=2) as sepool,
            tc.tile_pool(name="mp", bufs=2, space="PSUM") as mppool,
            tc.tile_pool(name="dp", bufs=dp_bufs, space="PSUM") as dppool,
            tc.tile_pool(name="r1p", bufs=1, space="PSUM") as r1ppool,
            tc.tile_pool(name="r2p", bufs=2, space="PSUM") as r2ppool,
        ):
            # ---- constants into SBUF ----
            wbf = cpool.tile([128, nbcol, 128], bf16, name="wbf", tag="wbf")
            wfc = cpool.tile([128, 34], f32, name="wfc", tag="wfc")
            wsm = cpool.tile([16, 256], f32, name="wsm", tag="wsm")
            if not mask_dma:
                esm = cpool.tile([8, 2, 128], fp8, name="esm", tag="esm")
                nc.sync.dma_start(esm[:], esm_d[:])
            nc.sync.dma_start(wfc[:], wfc_d[:])
            nc.sync.dma_start(wsm[:], wsm_d[:])
            if det_fp8:
                wf8 = cpool.tile([128, 2, 2, 128], fp8, name="wf8", tag="wf8")
                nc.sync.dma_start(wf8[:], wf8_d[:])
            if repeat:
                nc.sync.dma_start(wbf[:], wbf_d[:])

            co = 0 if det_fp8 else 4
            if not det_fp8:
                wdet_bf = [[wbf[:, u * 2 + t] for t in range(2)] for u in range(2)]
            # rec1 col block: (kt, u, t)
            wr1 = [[[wbf[:, co + (kt * 2 + u) * 2 + t] for t in range(2)]
                    for u in range(2)] for kt in range(2)]
            wr2 = [[wbf[:, co + 8 + u * 2 + h] for h in range(2)] for u in range(2)]
            wfc1 = [wfc[:, u * 16:(u + 1) * 16] for u in range(2)]
            abias = [wfc[:, 32 + h:33 + h] for h in range(2)]
            wfc2 = [wsm[:, h * 128:(h + 1) * 128] for h in range(2)]

            st = {}      # b -> (xm_sb, xq_sb, mc)
            ot = {}      # (b, h) -> out SBUF tile

            def p1_head(b):
                if mask_dma:
                    m16_sb = m16pool.tile([128, HW], fp8, name=f"m16_b{b}",
                                          tag="m16")
                    nc.sync.dma_start(
                        m16_sb[:].rearrange("(g r) n -> g r n", r=8),
                        m16_d[b].broadcast_to([16, 8, HW]))
                else:
                    m16_sb = m16pool.tile([8, 2, HW], fp8, name=f"m16_b{b}",
                                          tag="m16")
                    nc.sync.dma_start(m16_sb[:], m16_d[b])
                x_sb = xpool.tile([128, 2, HW], bf16, name=f"x_b{b}", tag="x")
                # quarter-chunks per sub-row so tile 0's STT unblocks early
                NQ = HW // 4
                for u in range(2):
                    nc.sync.dma_start(x_sb[:, u, 0:NQ], x_d[b, :, u, 0:NQ])
                if b == 0 and not repeat:
                    nc.sync.dma_start(wbf[:], wbf_d[:])
                for ck in range(1, 4):
                    for u in range(2):
                        nc.sync.dma_start(
                            x_sb[:, u, ck * NQ:(ck + 1) * NQ],
                            x_d[b, :, u, ck * NQ:(ck + 1) * NQ])

                xm_sb = xmpool.tile([128, 2, HW], bf16, name=f"xm_b{b}", tag="xm")
                xq_sb = (xqpool.tile([128, 2, HW], fp8, name=f"xq_b{b}", tag="xq")
                         if det_fp8 else None)
                ysum = ypool.tile([128, 2, 8], f32, name=f"ysum_b{b}", tag="ysum")
                st[b] = dict(m16=m16_sb, x=x_sb, xm=xm_sb, xq=xq_sb, ysum=ysum)

            def p1_tile(b, j):
                s = st[b]
                n0 = j * NT
                if mask_dma:
                    mp = s["m16"][:, n0:n0 + NT]
                else:
                    mpt = mppool.tile([128, NT], f32, name=f"mp_b{b}j{j}",
                                      tag="mp")
                    nc.tensor.matmul(
                        mpt[:], esm[:], s["m16"][:, :, n0:n0 + NT],
                        start=True, stop=True, perf_mode=DR,
                    )
                    mp = mpt[:]
                for u in range(2):
                    eng = nc.vector
                    eng.scalar_tensor_tensor(
                        out=s["xm"][:, u, n0:n0 + NT],
                        in0=s["x"][:, u, n0:n0 + NT],
                        scalar=0.0,
                        in1=mp,
                        op0=BYPASS,
                        op1=MULT,
                        accum_out=s["ysum"][:, u, j:j + 1],
                    )
                if det_fp8:
                    xe = xq_b0 if (b == 0 and xq_b0) else xq_eng
                    sl = (slice(None), slice(None), slice(n0, n0 + NT))
                    if xe == "pool":
                        nc.gpsimd.tensor_copy(s["xq"][sl], s["xm"][sl])
                    elif xe == "act":
                        nc.scalar.activation(s["xq"][sl], s["xm"][sl], COPY)
                    elif xe == "dve":
                        nc.vector.tensor_copy(s["xq"][sl], s["xm"][sl])
                    else:  # split: ACT u0, DVE u1
                        nc.scalar.activation(
                            s["xq"][:, 0, n0:n0 + NT], s["xm"][:, 0, n0:n0 + NT],
                            COPY)
                        nc.vector.tensor_copy(
                            s["xq"][:, 1, n0:n0 + NT], s["xm"][:, 1, n0:n0 + NT])

            def p1_tail(b):
                s = st[b]
                ysum = s["ysum"]
                if debug:
                    nc.sync.dma_start(dxm_d[b], s["xm"][:].bitcast(f32))
                # SE chain -> per-channel output scale mc[h]
                nc.vector.reduce_sum(ysum[:, :, 7:8], ysum[:, :, 0:NTILES],
                                     axis=mybir.AxisListType.X)
                fc1p = r2ppool.tile([16, 1], f32, name=f"fc1p_b{b}", tag="r2p")
                nc.tensor.matmul(fc1p[:], wfc1[0][:], ysum[:, 0, 7:8],
                                 start=True, stop=False)
                nc.tensor.matmul(fc1p[:], wfc1[1][:], ysum[:, 1, 7:8],
                                 start=False, stop=True)
                h1 = sepool.tile([16, 1], f32, name=f"h1_b{b}", tag="h1")
                nc.scalar.activation(h1[:], fc1p[:], RELU)
                mc = []
                for h in range(2):
                    scp = r2ppool.tile([128, 1], f32, name=f"scp_b{b}h{h}", tag="r2p")
                    nc.tensor.matmul(scp[:], wfc2[h][:], h1[:],
                                     start=True, stop=True)
                    ssb = sepool.tile([128, 1], f32, name=f"ssb_b{b}h{h}", tag="ssb")
                    nc.scalar.activation(ssb[:], scp[:], SIGMOID)
                    mch = mcpool.tile([128, 1], f32, name=f"mc_b{b}h{h}", tag="mc")
                    if mc_dve:
                        # relu(ssb + abias) on DVE keeps ACT off the window path
                        nc.vector.tensor_scalar(
                            out=mch[:], in0=ssb[:], scalar1=abias[h][:],
                            scalar2=0.0,
                            op0=mybir.AluOpType.add, op1=mybir.AluOpType.max)
                    else:
                        nc.scalar.activation(mch[:], ssb[:], RELU,
                                             bias=abias[h][:])
                    mc.append(mch)
                if debug:
                    nc.sync.dma_start(dy_d[b], ysum[:])
                    for h in range(2):
                        nc.sync.dma_start(dmc_d[b, h], mc[h][:])
                s["mc"] = mc

            def det_block(b, j):
                """det GEMM -> sigmoid -> zh, for tile (b, j)."""
                s = st[b]
                n0 = j * NT
                sg = sgpool.tile([128, 2, NT], bf16, name=f"sg_b{b}j{j}", tag="sg")
                dp = (None if sig2 else
                      dppool.tile([128, 2, 512], f32, name=f"dp_b{b}j{j}",
                                  tag="dp"))
                for t in range(2):
                    dpt = (dppool.tile([128, 512], f32, name=f"dp_b{b}j{j}t{t}",
                                       tag="dp")
                           if sig2 else dp[:, t])
                    if det_fp8:
                        nc.tensor.matmul(
                            dpt[:, 0:NT], wf8[:, t], s["xq"][:, :, n0:n0 + NT],
                            start=True, stop=True, perf_mode=DR,
                        )
                    else:
                        for u in range(2):
                            nc.tensor.matmul(
                                dpt[:, 0:NT], wdet_bf[u][t][:],
                                s["xm"][:, u, n0:n0 + NT],
                                start=(u == 0), stop=(u == 1),
                            )
                    if sig2:
                        nc.scalar.activation(sg[:, t, :], dpt[:, 0:NT], SIGMOID)
                if not sig2:
                    nc.scalar.activation(sg[:], dp[:, :, 0:NT], SIGMOID)
                zh = zhpool.tile([128, 2, NT], bf16, name=f"zh_b{b}j{j}", tag="zh")
                nc.vector.tensor_tensor(zh[:], sg[:], s["xm"][:, :, n0:n0 + NT],
                                        MULT)
                s[("zh", j)] = zh

            def rec1_block(b, j):
                s = st[b]
                n0 = j * NT
                zh = s.pop(("zh", j))
                r1p = r1ppool.tile([128, 2, 512], f32, name=f"r1p_b{b}j{j}",
                                   tag="r1p")
                for t in range(2):
                    kts = [(wr1[0][0][t], s["xm"][:, 0, n0:n0 + NT]),
                           (wr1[0][1][t], s["xm"][:, 1, n0:n0 + NT]),
                           (wr1[1][0][t], zh[:, 0, :]),
                           (wr1[1][1][t], zh[:, 1, :])]
                    for k, (wk, mk) in enumerate(kts):
                        nc.tensor.matmul(r1p[:, t, 0:NT], wk[:], mk,
                                         start=(k == 0), stop=(k == 3))
                r1sb = r1pool.tile([128, 2, NT], bf16, name=f"r1_b{b}j{j}", tag="r1")
                nc.scalar.activation(r1sb[:], r1p[:, :, 0:NT], RELU)
                s[("r1", j)] = r1sb

            def rec2_block(b, j):
                s = st[b]
                n0 = j * NT
                r1sb = s.pop(("r1", j))
                mc = s["mc"]
                for h in range(2):
                    r2p = r2ppool.tile([128, NT], f32, name=f"r2p_b{b}h{h}j{j}",
                                       tag="r2p")
                    for u in range(2):
                        nc.tensor.matmul(r2p[:], wr2[u][h][:], r1sb[:, u, :],
                                         start=(u == 0), stop=(u == 1))
                    if j % 2 == 0:
                        ot[(b, h)] = opool.tile([128, 2 * NT], odt,
                                                name=f"o_b{b}h{h}j{j}", tag=f"o{h}")
                    o = ot[(b, h)][:, (j % 2) * NT:(j % 2 + 1) * NT]
                    if h == 0:
                        nc.vector.tensor_scalar_mul(o, r2p[:], mc[0][:])
                    else:
                        nc.scalar.activation(o, r2p[:], COPY, scale=mc[1][:])
                    if j % 2 == 1 or j == NTILES - 1:
                        w = (j % 2 + 1) * NT
                        nc.sync.dma_start(
                            out_d[b, h, :, n0 - (j % 2) * NT:n0 + NT],
                            ot[(b, h)][:, 0:w])

            import contextlib as _ctxlib
            rep_cm = (tc.For_i(0, repeat, 1,
                               hint_engines=(mybir.EngineType.PE,
                                             mybir.EngineType.DVE,
                                             mybir.EngineType.Activation,
                                             mybir.EngineType.SP,
                                             mybir.EngineType.Pool))
                      if repeat else _ctxlib.nullcontext())
            with rep_cm:
                if sched == "pipe":
                    # 2-deep software pipeline over flat (b, j) windows:
                    # window (b,j) emits det(b,j+1), rec1(b,j), rec2(b,j-1)
                    # plus one phase1 tile of image b+1.
                    PRO = NTILES  # image-0 phase1 tiles before windows
                    DRE = 1       # rec2 emission delay in windows
                    for rep in range(unroll):
                        p1_head(0)
                        for j in range(PRO):
                            p1_tile(0, j)
                        if PRO == NTILES:
                            p1_tail(0)
                        det_block(0, 0)
                        units = [(b, j) for b in range(BC)
                                 for j in range(NTILES)]
                        for w, (b, j) in enumerate(units):
                            if w + 1 < len(units):
                                det_block(*units[w + 1])
                            rec1_block(b, j)
                            if w >= DRE:
                                rec2_block(*units[w - DRE])
                            if b == 0 and j + PRO < NTILES:
                                p1_tile(0, j + PRO)
                                if j + PRO == NTILES - 1:
                                    p1_tail(0)
                            if b + 1 < BC:
                                if j == 0:
                                    p1_head(b + 1)
                                p1_tile(b + 1, j)
                                if j == NTILES - 1:
                                    p1_tail(b + 1)
                        for w in range(len(units) - DRE, len(units)):
                            rec2_block(*units[w])
                        st.clear()
                else:
                    for b in range(BC):
                        p1_head(b)
                        for j in range(NTILES):
                            p1_tile(b, j)
                        p1_tail(b)
                        for j in range(NTILES):
                            det_block(b, j)
                            rec1_block(b, j)
                            rec2_block(b, j)
                        del st[b]

    _split_multiwaits(nc, mybir)
    return nc


def _jax_perm_cpu(num_chunks: int) -> np.ndarray:
    """jax.random.permutation(key(1234), num_chunks) on the CPU backend.

    Run in a JAX_PLATFORMS=cpu subprocess: in this process jax may be bound
    to an accelerator backend that cannot lower the shuffle's sort op.
    """
    import os
    import subprocess
    import sys
    import tempfile

    import jax

    sp = os.path.dirname(os.path.dirname(jax.__file__))
    code = (
        "import sys, numpy as np, jax\n"
        f"perm = np.asarray(jax.random.permutation(jax.random.key(1234), {num_chunks}))\n"
        "np.save(sys.argv[1], perm)\n"
    )
    with tempfile.TemporaryDirectory() as td:
        path = os.path.join(td, "perm.npy")
        env = dict(os.environ, JAX_PLATFORMS="cpu", PYTHONPATH=sp)
        env.pop("TRN_TERMINAL_POOL_IPS", None)
        subprocess.run([sys.executable, "-c", code, path], env=env, check=True)
        return np.load(path)


def _mask16(rate: int) -> np.ndarray:
    """Per-image [16, HW] fp8 keep-mask in channel-group space."""
    n = B * C * HW
    num_chunks = math.ceil(n * 4 / 1472)
    num_lossy = int(math.ceil(num_chunks * (rate / 100)))
    perm = _jax_perm_cpu(num_chunks)
    keep = np.ones((num_chunks,), np.float32)
    if num_lossy > 0:
        keep[perm[:num_lossy]] = 0.0
    bg = np.arange(B, dtype=np.int64)
    qq = np.arange(QG, dtype=np.int64)
    pp = np.arange(HW, dtype=np.int64)
    u = (bg[:, None, None] * HW + pp[None, None, :]) * QG + qq[None, :, None]
    return keep[u // UPC].astype(ml_dtypes.float8_e4m3)


def _prep_in_maps(inputs, det_fp8=True, mask_dma=False):
    x = np.asarray(inputs["x"], dtype=np.float32)
    rate = int(np.asarray(inputs["Packet_Loss_Rate"]))
    fc1 = np.asarray(inputs["fc1_w"], dtype=np.float32)
    fc2 = np.asarray(inputs["fc2_w"], dtype=np.float32)
    thr = float(np.asarray(inputs["threshold"], dtype=np.float32).reshape(-1)[0])
    detw = np.asarray(inputs["detect_w"], dtype=np.float32)
    r1w = np.asarray(inputs["rec1_w"], dtype=np.float32)
    r2w = np.asarray(inputs["rec2_w"], dtype=np.float32)
    aw = np.asarray(inputs["adapt_w"], dtype=np.float32)

    bf = ml_dtypes.bfloat16
    f8 = ml_dtypes.float8_e4m3

    # x: [B, C, HW] -> [B, 128, 2, HW] bf16 (c = 2p + u)
    xr = np.ascontiguousarray(
        x.reshape(B, 128, 2, HW).astype(bf))

    # bf16 weight blob [128, ncol, 128]
    detT = detw.T.reshape(128, 2, 128, 2)               # [p, u, q, t]
    r1T = r1w.T.reshape(2, 128, 2, 128, 2)              # [kt, p, u, q, t]
    r2T = r2w.T.reshape(128, 2, 2, 128)                 # [p, u, h, q]
    blocks = []
    if not det_fp8:
        blocks.append(detT.transpose(0, 1, 3, 2).reshape(128, 4, 128))
    blocks.append(r1T.transpose(1, 0, 2, 4, 3).reshape(128, 8, 128))
    blocks.append(r2T.reshape(128, 4, 128))
    wbf = np.ascontiguousarray(np.concatenate(blocks, axis=1).astype(bf))

    # E2 expansion: [8, 2, 128], 1 iff group(2q) == 2*pg + ug
    pg = np.arange(8)
    ug = np.arange(2)
    q = np.arange(128)
    esm = ((2 * pg[:, None, None] + ug[None, :, None]) ==
           (q[None, None, :] // 8)).astype(f8)

    wfc = np.zeros((128, 34), np.float32)
    fc1T = (fc1.T / HW).reshape(128, 2, 16)             # [p, u, m]
    wfc[:, 0:16] = fc1T[:, 0, :]
    wfc[:, 16:32] = fc1T[:, 1, :]
    ab = (rate * aw[:, 0] - thr).astype(np.float32)
    wfc[:, 32] = ab[0:128]
    wfc[:, 33] = ab[128:256]

    wsm = np.ascontiguousarray(fc2.T.astype(np.float32))  # [16, 256]

    if mask_dma:
        m16 = _mask16(rate).reshape(B, 16, 1, HW)
    else:
        m16 = _mask16(rate).reshape(B, 8, 2, HW)

    in_maps = []
    for c in range(NCORES):
        m = {
            "x": xr[c * BC:(c + 1) * BC],
            "m16": m16[c * BC:(c + 1) * BC],
            "wbf": wbf, "esm": esm, "wfc": wfc, "wsm": wsm,
        }
        if det_fp8:
            m["wf8"] = np.ascontiguousarray(
                detT.transpose(0, 3, 1, 2).astype(f8))  # [p, t, u, q]
        if mask_dma:
            del m["esm"]
        in_maps.append(m)
    return in_maps


BUILD_KW = dict(det_fp8=True, xq_eng="pool", sched="pipe", out_bf16=True,
                sig2=True, mc_dve=True, mask_dma=False)


def kernel(**inputs) -> np.ndarray:
    from concourse.bass_utils import run_bass_kernel_spmd

    kw = _CACHE.get("kw", BUILD_KW)
    in_maps = _prep_in_maps(inputs, det_fp8=kw.get("det_fp8", False),
                            mask_dma=kw.get("mask_dma", False))
    if "nc" not in _CACHE:
        _CACHE["nc"] = _build(**kw)
    nc = _CACHE["nc"]
    last_err = None
    for _attempt in range(3):
        try:
            res = run_bass_kernel_spmd(nc, in_maps, core_ids=list(range(NCORES)))
            break
        except Exception as e:  # transient axon/device hiccups: retry
            last_err = e
    else:
        raise last_err
    out = np.stack([res.results[c]["out"] for c in range(NCORES)], axis=0)
    return out.reshape(B, C, H, W).astype(np.float32)


# revision 3
# speedup vs baseline: 1.0440x; 1.0440x over previous
"""EnhancedChannelFilter Trainium2 kernel (pair-interleaved layout).

Full inputs in, full outputs out. Pure data-parallel over 8 NeuronCores
(4 images each). Channels are pair-interleaved on SBUF: channel c = 2p+u
lives at partition p, sub-row u, so tiles are [128, 2, pixels].

Per core, per image:
  1. x is shipped from the host as bf16 [128, 2, HW] (layout/precision prep;
     end-to-end numerics validated at rel err 8.9e-3 vs the 2e-2 budget).
  2. Packet-loss mask shipped as fp8 [8, 2, HW] in channel-group space,
     expanded on the PE by a 0/1 DoubleRow matmul to [128, NT] per n-tile
     (mask is constant across channel pairs, so one expand serves both
     sub-rows).
  3. xm = x * mask fused with the SE row-sum via DVE STT accum_out (the
     f32 accumulation happens pre-bf16-rounding, keeping the SE path exact).
  4. SE chain (fc1 -> relu -> fc2 -> sigmoid -> +bias -> relu) on PE/ACT/DVE;
     1/HW and rate*adapt_w - threshold folded into host-packed weights.
  5. det/rec1/rec2 GEMMs in bf16 (1 col/cycle; fp8 DoubleRow for det was
     measured SLOWER on hardware despite the cost model -- LDWEIGHTS pays
     +72% in DR mode); sigmoid/relu PSUM evictions on ACT, zh =
     sigmoid(det)*xm on DVE, final per-channel scale split DVE/ACT, bf16
     output tiles with paired out DMAs (host upcasts to f32).

Scheduling: a 2-deep software pipeline over flat (image, tile) windows --
window w emits det(w+1), rec1(w), rec2(w-1) plus one phase-1 tile of the
next image -- so the PE never waits on the det->sigmoid->zh chain and all
four engines run concurrently at ~90% of the PE's 2.5us/window.
"""

import math

import numpy as np
import ml_dtypes

B, C, H, W = 32, 256, 56, 56
HW = H * W              # 3136
NCORES = 8
BC = B // NCORES        # images per core
NT = 448                # pixels per n-tile
NTILES = HW // NT       # 7
EPC = 1472 // 4         # f32 elements per packet chunk (368)
QG = 16                 # channel-group size: gcd(EPC, C)
UPC = EPC // QG         # 23 channel-group-units per chunk

_CACHE: dict = {}


# ---------------------------------------------------------------------------
# Workaround: this walrus build enforces 1 sync wait per instruction (2 for
# EventSemaphore), but the Tile framework attaches several to its exit drain.
# Splitting extra waits onto dedicated same-engine NOPs placed immediately
# before the instruction is semantically identical.
# ---------------------------------------------------------------------------
def _split_multiwaits(nc, mybir):
    n = 0
    for bb in nc.m.functions[0].blocks:
        lst = bb.instructions
        for inst in list(lst):
            si = inst.sync_info
            if si is None or not si.on_wait:
                continue
            cap = 2 if isinstance(inst, mybir.InstEventSemaphore) else 1
            waits = list(si.on_wait)
            if len(waits) <= cap:
                continue
            eng = nc.engines[inst.engine]
            extra = []
            for wt in waits[:-cap]:
                nop = eng.nop(nofuse=True).ins
                nop.sync_info = mybir.SyncInfo(on_wait=[wt], on_update=[])
                nc.cur_bb.bb.instructions.remove(nop)
                extra.append(nop)
            si.on_wait = waits[-cap:]
            idx = lst.index(inst)
            lst[idx:idx] = extra
            n += 1
    return n


def _build(debug=False, repeat=0, det_fp8=False, xq_eng="pool", sched="pipe",
           out_bf16=True, sig2=True, mc_dve=True, xq_b0=None, unroll=1,
           mask_dma=False):
    import concourse.bass as bass
    import concourse.tile as tile
    import concourse.mybir as mybir

    f32 = mybir.dt.float32
    bf16 = mybir.dt.bfloat16
    fp8 = mybir.dt.float8e4
    DR = mybir.MatmulPerfMode.DoubleRow
    MULT = mybir.AluOpType.mult
    BYPASS = mybir.AluOpType.bypass
    SIGMOID = mybir.ActivationFunctionType.Sigmoid
    COPY = mybir.ActivationFunctionType.Copy
    RELU = mybir.ActivationFunctionType.Relu

    dp_bufs = (2 if mask_dma else 1) if not sig2 else 2
    nc = bass.Bass("TRN2", target_bir_lowering=False, debug=False)

    x_d = nc.dram_tensor("x", [BC, 128, 2, HW], bf16, kind="ExternalInput").ap()
    if mask_dma:
        m16_d = nc.dram_tensor("m16", [BC, 16, 1, HW], fp8,
                               kind="ExternalInput").ap()
    else:
        m16_d = nc.dram_tensor("m16", [BC, 8, 2, HW], fp8,
                               kind="ExternalInput").ap()
    # bf16 GEMM weights: [128, ncol, 128] with col blocks
    #   det (u,t) 4 cols (absent when det_fp8), rec1 (kt,u,t) 8, rec2 (u,h) 4
    nbcol = (0 if det_fp8 else 4) + 8 + 4
    wbf_d = nc.dram_tensor("wbf", [128, nbcol, 128], bf16, kind="ExternalInput").ap()
    if det_fp8:
        wf8_d = nc.dram_tensor("wf8", [128, 2, 2, 128], fp8, kind="ExternalInput").ap()
    if not mask_dma:
        esm_d = nc.dram_tensor("esm", [8, 2, 128], fp8,
                               kind="ExternalInput").ap()
    # f32 smalls: fc1 (u) 2x16 cols then abias (h) 2x1
    wfc_d = nc.dram_tensor("wfc", [128, 34], f32, kind="ExternalInput").ap()
    wsm_d = nc.dram_tensor("wsm", [16, 256], f32, kind="ExternalInput").ap()
    odt = bf16 if out_bf16 else f32
    out_d = nc.dram_tensor("out", [BC, 2, 128, HW], odt, kind="ExternalOutput").ap()
    if debug:
        dxm_d = nc.dram_tensor("dxm", [BC, 128, 2, HW], f32, kind="ExternalOutput").ap()
        dsg_d = nc.dram_tensor("dsg", [BC, 128, 2, HW], f32, kind="ExternalOutput").ap()
        dmc_d = nc.dram_tensor("dmc", [BC, 2, 128, 1], f32, kind="ExternalOutput").ap()
        dy_d = nc.dram_tensor("dy", [BC, 128, 2, 8], f32, kind="ExternalOutput").ap()

    with tile.TileContext(nc) as tc:
        with (
            tc.tile_pool(name="consts", bufs=1) as cpool,
            tc.tile_pool(name="xin", bufs=2) as xpool,
            tc.tile_pool(name="xm", bufs=2) as xmpool,
            tc.tile_pool(name="xq", bufs=2) as xqpool,
            tc.tile_pool(name="m16", bufs=2) as m16pool,
            tc.tile_pool(name="sg", bufs=3) as sgpool,
            tc.tile_pool(name="zh", bufs=3) as zhpool,
            tc.tile_pool(name="r1", bufs=6) as r1pool,
            tc.tile_pool(name="osb", bufs=2) as opool,
            tc.tile_pool(name="ysum", bufs=2) as ypool,
            tc.tile_pool(name="mch", bufs=4) as mcpool,
            tc.tile_pool(name="sesb", bufs=2) as sepool,
            tc.tile_pool(name="mp", bufs=2, space="PSUM") as mppool,
            tc.tile_pool(name="dp", bufs=dp_bufs, space="PSUM") as dppool,
            tc.tile_pool(name="r1p", bufs=1, space="PSUM") as r1ppool,
            tc.tile_pool(name="r2p", bufs=2, space="PSUM") as r2ppool,
        ):
            # ---- constants into SBUF ----
            wbf = cpool.tile([128, nbcol, 128], bf16, name="wbf", tag="wbf")
            wfc = cpool.tile([128, 34], f32, name="wfc", tag="wfc")
            wsm = cpool.tile([16, 256], f32, name="wsm", tag="wsm")
            if not mask_dma:
                esm = cpool.tile([8, 2, 128], fp8, name="esm", tag="esm")
                nc.sync.dma_start(esm[:], esm_d[:])
            nc.sync.dma_start(wfc[:], wfc_d[:])
            nc.sync.dma_start(wsm[:], wsm_d[:])
            if det_fp8:
                wf8 = cpool.tile([128, 2, 2, 128], fp8, name="wf8", tag="wf8")
                nc.sync.dma_start(wf8[:], wf8_d[:])
            if repeat:
                nc.sync.dma_start(wbf[:], wbf_d[:])

            co = 0 if det_fp8 else 4
            if not det_fp8:
                wdet_bf = [[wbf[:, u * 2 + t] for t in range(2)] for u in range(2)]
            # rec1 col block: (kt, u, t)
            wr1 = [[[wbf[:, co + (kt * 2 + u) * 2 + t] for t in range(2)]
                    for u in range(2)] for kt in range(2)]
            wr2 = [[wbf[:, co + 8 + u * 2 + h] for h in range(2)] for u in range(2)]
            wfc1 = [wfc[:, u * 16:(u + 1) * 16] for u in range(2)]
            abias = [wfc[:, 32 + h:33 + h] for h in range(2)]
            wfc2 = [wsm[:, h * 128:(h + 1) * 128] for h in range(2)]

            st = {}      # b -> (xm_sb, xq_sb, mc)
            ot = {}      # (b, h) -> out SBUF tile

            def p1_head(b):
                if mask_dma:
                    m16_sb = m16pool.tile([128, HW], fp8, name=f"m16_b{b}",
                                          tag="m16")
                    nc.sync.dma_start(
                        m16_sb[:].rearrange("(g r) n -> g r n", r=8),
                        m16_d[b].broadcast_to([16, 8, HW]))
                else:
                    m16_sb = m16pool.tile([8, 2, HW], fp8, name=f"m16_b{b}",
                                          tag="m16")
                    nc.sync.dma_start(m16_sb[:], m16_d[b])
                x_sb = xpool.tile([128, 2, HW], bf16, name=f"x_b{b}", tag="x")
                # quarter-chunks per sub-row so tile 0's STT unblocks early
                NQ = HW // 4
                for u in range(2):
                    nc.sync.dma_start(x_sb[:, u, 0:NQ], x_d[b, :, u, 0:NQ])
                if b == 0 and not repeat:
                    nc.sync.dma_start(wbf[:], wbf_d[:])
                for ck in range(1, 4):
                    for u in range(2):
                        nc.sync.dma_start(
                            x_sb[:, u, ck * NQ:(ck + 1) * NQ],
                            x_d[b, :, u, ck * NQ:(ck + 1) * NQ])

                xm_sb = xmpool.tile([128, 2, HW], bf16, name=f"xm_b{b}", tag="xm")
                xq_sb = (xqpool.tile([128, 2, HW], fp8, name=f"xq_b{b}", tag="xq")
                         if det_fp8 else None)
                ysum = ypool.tile([128, 2, 8], f32, name=f"ysum_b{b}", tag="ysum")
                st[b] = dict(m16=m16_sb, x=x_sb, xm=xm_sb, xq=xq_sb, ysum=ysum)

            def p1_tile(b, j):
                s = st[b]
                n0 = j * NT
                if mask_dma:
                    mp = s["m16"][:, n0:n0 + NT]
                else:
                    mpt = mppool.tile([128, NT], f32, name=f"mp_b{b}j{j}",
                                      tag="mp")
                    nc.tensor.matmul(
                        mpt[:], esm[:], s["m16"][:, :, n0:n0 + NT],
                        start=True, stop=True, perf_mode=DR,
                    )
                    mp = mpt[:]
                for u in range(2):
                    eng = nc.vector
                    eng.scalar_tensor_tensor(
                        out=s["xm"][:, u, n0:n0 + NT],
                        in0=s["x"][:, u, n0:n0 + NT],
                        scalar=0.0,
                        in1=mp,
                        op0=BYPASS,
                        op1=MULT,
                        accum_out=s["ysum"][:, u, j:j + 1],
                    )
                if det_fp8:
                    xe = xq_b0 if (b == 0 and xq_b0) else xq_eng
                    sl = (slice(None), slice(None), slice(n0, n0 + NT))
                    if xe == "pool":
                        nc.gpsimd.tensor_copy(s["xq"][sl], s["xm"][sl])
                    elif xe == "act":
                        nc.scalar.activation(s["xq"][sl], s["xm"][sl], COPY)
                    elif xe == "dve":
                        nc.vector.tensor_copy(s["xq"][sl], s["xm"][sl])
                    else:  # split: ACT u0, DVE u1
                        nc.scalar.activation(
                            s["xq"][:, 0, n0:n0 + NT], s["xm"][:, 0, n0:n0 + NT],
                            COPY)
                        nc.vector.tensor_copy(
                            s["xq"][:, 1, n0:n0 + NT], s["xm"][:, 1, n0:n0 + NT])

            def p1_tail(b):
                s = st[b]
                ysum = s["ysum"]
                if debug:
                    nc.sync.dma_start(dxm_d[b], s["xm"][:].bitcast(f32))
                # SE chain -> per-channel output scale mc[h]
                nc.vector.reduce_sum(ysum[:, :, 7:8], ysum[:, :, 0:NTILES],
                                     axis=mybir.AxisListType.X)
                fc1p = r2ppool.tile([16, 1], f32, name=f"fc1p_b{b}", tag="r2p")
                nc.tensor.matmul(fc1p[:], wfc1[0][:], ysum[:, 0, 7:8],
                                 start=True, stop=False)
                nc.tensor.matmul(fc1p[:], wfc1[1][:], ysum[:, 1, 7:8],
                                 start=False, stop=True)
                h1 = sepool.tile([16, 1], f32, name=f"h1_b{b}", tag="h1")
                nc.scalar.activation(h1[:], fc1p[:], RELU)
                mc = []
                for h in range(2):
                    scp = r2ppool.tile([128, 1], f32, name=f"scp_b{b}h{h}", tag="r2p")
                    nc.tensor.matmul(scp[:], wfc2[h][:], h1[:],
                                     start=True, stop=True)
                    ssb = sepool.tile([128, 1], f32, name=f"ssb_b{b}h{h}", tag="ssb")
                    nc.scalar.activation(ssb[:], scp[:], SIGMOID)
                    mch = mcpool.tile([128, 1], f32, name=f"mc_b{b}h{h}", tag="mc")
                    if mc_dve:
                        # relu(ssb + abias) on DVE keeps ACT off the window path
                        nc.vector.tensor_scalar(
                            out=mch[:], in0=ssb[:], scalar1=abias[h][:],
                            scalar2=0.0,
                            op0=mybir.AluOpType.add, op1=mybir.AluOpType.max)
                    else:
                        nc.scalar.activation(mch[:], ssb[:], RELU,
                                             bias=abias[h][:])
                    mc.append(mch)
                if debug:
                    nc.sync.dma_start(dy_d[b], ysum[:])
                    for h in range(2):
                        nc.sync.dma_start(dmc_d[b, h], mc[h][:])
                s["mc"] = mc

            def det_block(b, j):
                """det GEMM -> sigmoid -> zh, for tile (b, j)."""
                s = st[b]
                n0 = j * NT
                sg = sgpool.tile([128, 2, NT], bf16, name=f"sg_b{b}j{j}", tag="sg")
                dp = (None if sig2 else
                      dppool.tile([128, 2, 512], f32, name=f"dp_b{b}j{j}",
                                  tag="dp"))
                for t in range(2):
                    dpt = (dppool.tile([128, 512], f32, name=f"dp_b{b}j{j}t{t}",
                                       tag="dp")
                           if sig2 else dp[:, t])
                    if det_fp8:
                        nc.tensor.matmul(
                            dpt[:, 0:NT], wf8[:, t], s["xq"][:, :, n0:n0 + NT],
                            start=True, stop=True, perf_mode=DR,
                        )
                    else:
                        for u in range(2):
                            nc.tensor.matmul(
                                dpt[:, 0:NT], wdet_bf[u][t][:],
                                s["xm"][:, u, n0:n0 + NT],
                                start=(u == 0), stop=(u == 1),
                            )
                    if sig2:
                        nc.scalar.activation(sg[:, t, :], dpt[:, 0:NT], SIGMOID)
                if not sig2:
                    nc.scalar.activation(sg[:], dp[:, :, 0:NT], SIGMOID)
                zh = zhpool.tile([128, 2, NT], bf16, name=f"zh_b{b}j{j}", tag="zh")
                nc.vector.tensor_tensor(zh[:], sg[:], s["xm"][:, :, n0:n0 + NT],
                                        MULT)
                s[("zh", j)] = zh

            def rec1_block(b, j):
                s = st[b]
                n0 = j * NT
                zh = s.pop(("zh", j))
                r1p = r1ppool.tile([128, 2, 512], f32, name=f"r1p_b{b}j{j}",
                                   tag="r1p")
                for t in range(2):
                    kts = [(wr1[0][0][t], s["xm"][:, 0, n0:n0 + NT]),
                           (wr1[0][1][t], s["xm"][:, 1, n0:n0 + NT]),
                           (wr1[1][0][t], zh[:, 0, :]),
                           (wr1[1][1][t], zh[:, 1, :])]
                    for k, (wk, mk) in enumerate(kts):
                        nc.tensor.matmul(r1p[:, t, 0:NT], wk[:], mk,
                                         start=(k == 0), stop=(k == 3))
                r1sb = r1pool.tile([128, 2, NT], bf16, name=f"r1_b{b}j{j}", tag="r1")
                nc.scalar.activation(r1sb[:], r1p[:, :, 0:NT], RELU)
                s[("r1", j)] = r1sb

            def rec2_block(b, j):
                s = st[b]
                n0 = j * NT
                r1sb = s.pop(("r1", j))
                mc = s["mc"]
                for h in range(2):
                    r2p = r2ppool.tile([128, NT], f32, name=f"r2p_b{b}h{h}j{j}",
                                       tag="r2p")
                    for u in range(2):
                        nc.tensor.matmul(r2p[:], wr2[u][h][:], r1sb[:, u, :],
                                         start=(u == 0), stop=(u == 1))
                    if j % 2 == 0:
                        ot[(b, h)] = opool.tile([128, 2 * NT], odt,
                                                name=f"o_b{b}h{h}j{j}", tag=f"o{h}")
                    o = ot[(b, h)][:, (j % 2) * NT:(j % 2 + 1) * NT]
                    if h == 0:
                        nc.vector.tensor_scalar_mul(o, r2p[:], mc[0][:])
                    else:
                        nc.scalar.activation(o, r2p[:], COPY, scale=mc[1][:])
                    if j % 2 == 1 or j == NTILES - 1:
                        w = (j % 2 + 1) * NT
                        nc.sync.dma_start(
                            out_d[b, h, :, n0 - (j % 2) * NT:n0 + NT],
                            ot[(b, h)][:, 0:w])

            import contextlib as _ctxlib
            rep_cm = (tc.For_i(0, repeat, 1,
                               hint_engines=(mybir.EngineType.PE,
                                             mybir.EngineType.DVE,
                                             mybir.EngineType.Activation,
                                             mybir.EngineType.SP,
                                             mybir.EngineType.Pool))
                      if repeat else _ctxlib.nullcontext())
            with rep_cm:
                if sched == "pipe":
                    # 2-deep software pipeline over flat (b, j) windows:
                    # window (b,j) emits det(b,j+1), rec1(b,j), rec2(b,j-1)
                    # plus one phase1 tile of image b+1.
                    PRO = NTILES  # image-0 phase1 tiles before windows
                    DRE = 1       # rec2 emission delay in windows
                    for rep in range(unroll):
                        p1_head(0)
                        for j in range(PRO):
                            p1_tile(0, j)
                        if PRO == NTILES:
                            p1_tail(0)
                        det_block(0, 0)
                        units = [(b, j) for b in range(BC)
                                 for j in range(NTILES)]
                        for w, (b, j) in enumerate(units):
                            if w + 1 < len(units):
                                det_block(*units[w + 1])
                            rec1_block(b, j)
                            if w >= DRE:
                                rec2_block(*units[w - DRE])
                            if b == 0 and j + PRO < NTILES:
                                p1_tile(0, j + PRO)
                                if j + PRO == NTILES - 1:
                                    p1_tail(0)
                            if b + 1 < BC:
                                if j == 0:
                                    p1_head(b + 1)
                                p1_tile(b + 1, j)
                                if j == NTILES - 1:
                                    p1_tail(b + 1)
                        for w in range(len(units) - DRE, len(units)):
                            rec2_block(*units[w])
                        st.clear()
                else:
                    for b in range(BC):
                        p1_head(b)
                        for j in range(NTILES):
                            p1_tile(b, j)
                        p1_tail(b)
                        for j in range(NTILES):
                            det_block(b, j)
                            rec1_block(b, j)
                            rec2_block(b, j)
                        del st[b]

    _split_multiwaits(nc, mybir)
    return nc


def _jax_perm_cpu(num_chunks: int) -> np.ndarray:
    """jax.random.permutation(key(1234), num_chunks) on the CPU backend.

    Run in a JAX_PLATFORMS=cpu subprocess: in this process jax may be bound
    to an accelerator backend that cannot lower the shuffle's sort op.
    """
    import os
    import subprocess
    import sys
    import tempfile

    import jax

    sp = os.path.dirname(os.path.dirname(jax.__file__))
    code = (
        "import sys, numpy as np, jax\n"
        f"perm = np.asarray(jax.random.permutation(jax.random.key(1234), {num_chunks}))\n"
        "np.save(sys.argv[1], perm)\n"
    )
    with tempfile.TemporaryDirectory() as td:
        path = os.path.join(td, "perm.npy")
        env = dict(os.environ, JAX_PLATFORMS="cpu", PYTHONPATH=sp)
        env.pop("TRN_TERMINAL_POOL_IPS", None)
        subprocess.run([sys.executable, "-c", code, path], env=env, check=True)
        return np.load(path)


def _mask16(rate: int) -> np.ndarray:
    """Per-image [16, HW] fp8 keep-mask in channel-group space."""
    n = B * C * HW
    num_chunks = math.ceil(n * 4 / 1472)
    num_lossy = int(math.ceil(num_chunks * (rate / 100)))
    perm = _jax_perm_cpu(num_chunks)
    keep = np.ones((num_chunks,), np.float32)
    if num_lossy > 0:
        keep[perm[:num_lossy]] = 0.0
    bg = np.arange(B, dtype=np.int64)
    qq = np.arange(QG, dtype=np.int64)
    pp = np.arange(HW, dtype=np.int64)
    u = (bg[:, None, None] * HW + pp[None, None, :]) * QG + qq[None, :, None]
    return keep[u // UPC].astype(ml_dtypes.float8_e4m3)


def _prep_in_maps(inputs, det_fp8=False, mask_dma=False):
    x = np.asarray(inputs["x"], dtype=np.float32)
    rate = int(np.asarray(inputs["Packet_Loss_Rate"]))
    fc1 = np.asarray(inputs["fc1_w"], dtype=np.float32)
    fc2 = np.asarray(inputs["fc2_w"], dtype=np.float32)
    thr = float(np.asarray(inputs["threshold"], dtype=np.float32).reshape(-1)[0])
    detw = np.asarray(inputs["detect_w"], dtype=np.float32)
    r1w = np.asarray(inputs["rec1_w"], dtype=np.float32)
    r2w = np.asarray(inputs["rec2_w"], dtype=np.float32)
    aw = np.asarray(inputs["adapt_w"], dtype=np.float32)

    bf = ml_dtypes.bfloat16
    f8 = ml_dtypes.float8_e4m3

    # x: [B, C, HW] -> [B, 128, 2, HW] bf16 (c = 2p + u)
    xr = np.ascontiguousarray(
        x.reshape(B, 128, 2, HW).astype(bf))

    # bf16 weight blob [128, ncol, 128]
    detT = detw.T.reshape(128, 2, 128, 2)               # [p, u, q, t]
    r1T = r1w.T.reshape(2, 128, 2, 128, 2)              # [kt, p, u, q, t]
    r2T = r2w.T.reshape(128, 2, 2, 128)                 # [p, u, h, q]
    blocks = []
    if not det_fp8:
        blocks.append(detT.transpose(0, 1, 3, 2).reshape(128, 4, 128))
    blocks.append(r1T.transpose(1, 0, 2, 4, 3).reshape(128, 8, 128))
    blocks.append(r2T.reshape(128, 4, 128))
    wbf = np.ascontiguousarray(np.concatenate(blocks, axis=1).astype(bf))

    # E2 expansion: [8, 2, 128], 1 iff group(2q) == 2*pg + ug
    pg = np.arange(8)
    ug = np.arange(2)
    q = np.arange(128)
    esm = ((2 * pg[:, None, None] + ug[None, :, None]) ==
           (q[None, None, :] // 8)).astype(f8)

    wfc = np.zeros((128, 34), np.float32)
    fc1T = (fc1.T / HW).reshape(128, 2, 16)             # [p, u, m]
    wfc[:, 0:16] = fc1T[:, 0, :]
    wfc[:, 16:32] = fc1T[:, 1, :]
    ab = (rate * aw[:, 0] - thr).astype(np.float32)
    wfc[:, 32] = ab[0:128]
    wfc[:, 33] = ab[128:256]

    wsm = np.ascontiguousarray(fc2.T.astype(np.float32))  # [16, 256]

    if mask_dma:
        m16 = _mask16(rate).reshape(B, 16, 1, HW)
    else:
        m16 = _mask16(rate).reshape(B, 8, 2, HW)

    in_maps = []
    for c in range(NCORES):
        m = {
            "x": xr[c * BC:(c + 1) * BC],
            "m16": m16[c * BC:(c + 1) * BC],
            "wbf": wbf, "esm": esm, "wfc": wfc, "wsm": wsm,
        }
        if det_fp8:
            m["wf8"] = np.ascontiguousarray(
                detT.transpose(0, 3, 1, 2).astype(f8))  # [p, t, u, q]
        if mask_dma:
            del m["esm"]
        in_maps.append(m)
    return in_maps


BUILD_KW = dict(det_fp8=False, xq_eng="pool", sched="pipe", out_bf16=True,
                sig2=True, mc_dve=True, mask_dma=False)


def kernel(**inputs) -> np.ndarray:
    from concourse.bass_utils import run_bass_kernel_spmd

    kw = _CACHE.get("kw", BUILD_KW)
    in_maps = _prep_in_maps(inputs, det_fp8=kw.get("det_fp8", False),
                            mask_dma=kw.get("mask_dma", False))
    if "nc" not in _CACHE:
        _CACHE["nc"] = _build(**kw)
    nc = _CACHE["nc"]
    last_err = None
    for _attempt in range(3):
        try:
            res = run_bass_kernel_spmd(nc, in_maps, core_ids=list(range(NCORES)))
            break
        except Exception as e:  # transient axon/device hiccups: retry
            last_err = e
    else:
        raise last_err
    out = np.stack([res.results[c]["out"] for c in range(NCORES)], axis=0)
    return out.reshape(B, C, H, W).astype(np.float32)


# revision 4
# speedup vs baseline: 1.0542x; 1.0098x over previous
"""EnhancedChannelFilter Trainium2 kernel (pair-interleaved layout).

Full inputs in, full outputs out. Pure data-parallel over 8 NeuronCores
(4 images each). Channels are pair-interleaved on SBUF: channel c = 2p+u
lives at partition p, sub-row u, so tiles are [128, 2, pixels].

Per core, per image:
  1. x shipped bf16 [128, 2, HW]; packet-loss keep-mask shipped fp8
     PRE-EXPANDED to [128, HW] (exact 0/1 values; trading ~1.4 MB/core of
     spare DMA bandwidth for 28 PE mask-expand matmuls + their DoubleRow
     LDWEIGHTS reloads and a PSUM round-trip -- measured ~9 us faster on HW).
  2. xm = x * mask fused with the SE row-sum via DVE STT accum_out (the
     f32 accumulation happens pre-bf16-rounding, keeping the SE path exact).
  3. SE chain (fc1 -> relu -> fc2 -> sigmoid -> +bias -> relu) on PE/ACT/DVE;
     1/HW and rate*adapt_w - threshold folded into host-packed weights.
  4. det/rec1/rec2 GEMMs all bf16 (fp8 DoubleRow for det measured SLOWER on
     hardware despite the cost model -- LDWEIGHTS pays +72% in DR mode);
     sigmoid/relu PSUM evictions on ACT, zh = sigmoid(det)*xm on DVE, final
     per-channel scale split DVE/ACT, bf16 out tiles, 4-tile-grouped out
     DMAs (host upcasts to f32).

Scheduling: a 2-deep software pipeline over flat (image, tile) windows --
window w emits det(w+1), rec1(w), rec2(w-1) plus one phase-1 tile of the
next image -- so the PE never waits on the det->sigmoid->zh chain. DMA
issue is kept off the critical path (each dma_start costs ~650 ns of
sequencer + HWDGE time, serially per queue): one x DMA per sub-row, one
mask DMA per image, constants issued from the ACT queue.
"""

import math

import numpy as np
import ml_dtypes

B, C, H, W = 32, 256, 56, 56
HW = H * W              # 3136
NCORES = 8
BC = B // NCORES        # images per core
NT = 448                # pixels per n-tile
NTILES = HW // NT       # 7
EPC = 1472 // 4         # f32 elements per packet chunk (368)
QG = 16                 # channel-group size: gcd(EPC, C)
UPC = EPC // QG         # 23 channel-group-units per chunk

_CACHE: dict = {}


# ---------------------------------------------------------------------------
# Workaround: this walrus build enforces 1 sync wait per instruction (2 for
# EventSemaphore), but the Tile framework attaches several to its exit drain.
# Splitting extra waits onto dedicated same-engine NOPs placed immediately
# before the instruction is semantically identical.
# ---------------------------------------------------------------------------
def _split_multiwaits(nc, mybir):
    n = 0
    for bb in nc.m.functions[0].blocks:
        lst = bb.instructions
        for inst in list(lst):
            si = inst.sync_info
            if si is None or not si.on_wait:
                continue
            cap = 2 if isinstance(inst, mybir.InstEventSemaphore) else 1
            waits = list(si.on_wait)
            if len(waits) <= cap:
                continue
            eng = nc.engines[inst.engine]
            extra = []
            for wt in waits[:-cap]:
                nop = eng.nop(nofuse=True).ins
                nop.sync_info = mybir.SyncInfo(on_wait=[wt], on_update=[])
                nc.cur_bb.bb.instructions.remove(nop)
                extra.append(nop)
            si.on_wait = waits[-cap:]
            idx = lst.index(inst)
            lst[idx:idx] = extra
            n += 1
    return n


def _build(debug=False, repeat=0, det_fp8=False, xq_eng="pool", sched="pipe",
           out_bf16=True, sig2=True, mc_dve=True, xq_b0=None, unroll=1,
           mask_dma=True):
    import concourse.bass as bass
    import concourse.tile as tile
    import concourse.mybir as mybir

    f32 = mybir.dt.float32
    bf16 = mybir.dt.bfloat16
    fp8 = mybir.dt.float8e4
    DR = mybir.MatmulPerfMode.DoubleRow
    MULT = mybir.AluOpType.mult
    BYPASS = mybir.AluOpType.bypass
    SIGMOID = mybir.ActivationFunctionType.Sigmoid
    COPY = mybir.ActivationFunctionType.Copy
    RELU = mybir.ActivationFunctionType.Relu

    dp_bufs = (2 if mask_dma else 1) if not sig2 else 2
    nc = bass.Bass("TRN2", target_bir_lowering=False, debug=False)

    x_d = nc.dram_tensor("x", [BC, 128, 2, HW], bf16, kind="ExternalInput").ap()
    if mask_dma:
        m16_d = nc.dram_tensor("m16", [BC, 128, HW], fp8,
                               kind="ExternalInput").ap()
    else:
        m16_d = nc.dram_tensor("m16", [BC, 8, 2, HW], fp8,
                               kind="ExternalInput").ap()
    # bf16 GEMM weights: [128, ncol, 128] with col blocks
    #   det (u,t) 4 cols (absent when det_fp8), rec1 (kt,u,t) 8, rec2 (u,h) 4
    nbcol = (0 if det_fp8 else 4) + 8 + 4
    wbf_d = nc.dram_tensor("wbf", [128, nbcol, 128], bf16, kind="ExternalInput").ap()
    if det_fp8:
        wf8_d = nc.dram_tensor("wf8", [128, 2, 2, 128], fp8, kind="ExternalInput").ap()
    if not mask_dma:
        esm_d = nc.dram_tensor("esm", [8, 2, 128], fp8,
                               kind="ExternalInput").ap()
    # f32 smalls: fc1 (u) 2x16 cols then abias (h) 2x1
    wfc_d = nc.dram_tensor("wfc", [128, 34], f32, kind="ExternalInput").ap()
    wsm_d = nc.dram_tensor("wsm", [16, 256], f32, kind="ExternalInput").ap()
    odt = bf16 if out_bf16 else f32
    out_d = nc.dram_tensor("out", [BC, 2, 128, HW], odt, kind="ExternalOutput").ap()
    if debug:
        dxm_d = nc.dram_tensor("dxm", [BC, 128, 2, HW], f32, kind="ExternalOutput").ap()
        dsg_d = nc.dram_tensor("dsg", [BC, 128, 2, HW], f32, kind="ExternalOutput").ap()
        dmc_d = nc.dram_tensor("dmc", [BC, 2, 128, 1], f32, kind="ExternalOutput").ap()
        dy_d = nc.dram_tensor("dy", [BC, 128, 2, 8], f32, kind="ExternalOutput").ap()

    with tile.TileContext(nc) as tc:
        with (
            tc.tile_pool(name="consts", bufs=1) as cpool,
            tc.tile_pool(name="xin", bufs=3) as xpool,
            tc.tile_pool(name="xm", bufs=2) as xmpool,
            tc.tile_pool(name="xq", bufs=2) as xqpool,
            tc.tile_pool(name="m16", bufs=3) as m16pool,
            tc.tile_pool(name="sg", bufs=3) as sgpool,
            tc.tile_pool(name="zh", bufs=3) as zhpool,
            tc.tile_pool(name="r1", bufs=6) as r1pool,
            tc.tile_pool(name="osb", bufs=3) as opool,
            tc.tile_pool(name="ysum", bufs=2) as ypool,
            tc.tile_pool(name="mch", bufs=4) as mcpool,
            tc.tile_pool(name="sesb", bufs=2) as sepool,
            tc.tile_pool(name="mp", bufs=2, space="PSUM") as mppool,
            tc.tile_pool(name="dp", bufs=dp_bufs, space="PSUM") as dppool,
            tc.tile_pool(name="r1p", bufs=1, space="PSUM") as r1ppool,
            tc.tile_pool(name="r2p", bufs=2, space="PSUM") as r2ppool,
        ):
            # ---- constants into SBUF ----
            wbf = cpool.tile([128, nbcol, 128], bf16, name="wbf", tag="wbf")
            wfc = cpool.tile([128, 34], f32, name="wfc", tag="wfc")
            wsm = cpool.tile([16, 256], f32, name="wsm", tag="wsm")
            if not mask_dma:
                esm = cpool.tile([8, 2, 128], fp8, name="esm", tag="esm")
                nc.scalar.dma_start(esm[:], esm_d[:])
            nc.scalar.dma_start(wfc[:], wfc_d[:])
            nc.scalar.dma_start(wsm[:], wsm_d[:])
            if det_fp8:
                wf8 = cpool.tile([128, 2, 2, 128], fp8, name="wf8", tag="wf8")
                nc.sync.dma_start(wf8[:], wf8_d[:])
            if repeat:
                nc.sync.dma_start(wbf[:], wbf_d[:])

            co = 0 if det_fp8 else 4
            if not det_fp8:
                wdet_bf = [[wbf[:, u * 2 + t] for t in range(2)] for u in range(2)]
            # rec1 col block: (kt, u, t)
            wr1 = [[[wbf[:, co + (kt * 2 + u) * 2 + t] for t in range(2)]
                    for u in range(2)] for kt in range(2)]
            wr2 = [[wbf[:, co + 8 + u * 2 + h] for h in range(2)] for u in range(2)]
            wfc1 = [wfc[:, u * 16:(u + 1) * 16] for u in range(2)]
            abias = [wfc[:, 32 + h:33 + h] for h in range(2)]
            wfc2 = [wsm[:, h * 128:(h + 1) * 128] for h in range(2)]

            st = {}      # b -> (xm_sb, xq_sb, mc)
            ot = {}      # (b, h) -> out SBUF tile

            def p1_head(b):
                if mask_dma:
                    m16_sb = m16pool.tile([128, HW], fp8, name=f"m16_b{b}",
                                          tag="m16")
                    nc.sync.dma_start(m16_sb[:], m16_d[b])
                else:
                    m16_sb = m16pool.tile([8, 2, HW], fp8, name=f"m16_b{b}",
                                          tag="m16")
                    nc.sync.dma_start(m16_sb[:], m16_d[b])
                x_sb = xpool.tile([128, 2, HW], bf16, name=f"x_b{b}", tag="x")
                if b == 0 and not repeat:
                    nc.sync.dma_start(wbf[:], wbf_d[:])
                # one DMA per sub-row: fewer 650ns issue slots on the SP queue
                for u in range(2):
                    nc.sync.dma_start(x_sb[:, u, :], x_d[b, :, u, :])

                xm_sb = xmpool.tile([128, 2, HW], bf16, name=f"xm_b{b}", tag="xm")
                xq_sb = (xqpool.tile([128, 2, HW], fp8, name=f"xq_b{b}", tag="xq")
                         if det_fp8 else None)
                ysum = ypool.tile([128, 2, 8], f32, name=f"ysum_b{b}", tag="ysum")
                st[b] = dict(m16=m16_sb, x=x_sb, xm=xm_sb, xq=xq_sb, ysum=ysum)

            def p1_tile(b, j):
                s = st[b]
                n0 = j * NT
                if mask_dma:
                    mp = s["m16"][:, n0:n0 + NT]
                else:
                    mpt = mppool.tile([128, NT], f32, name=f"mp_b{b}j{j}",
                                      tag="mp")
                    nc.tensor.matmul(
                        mpt[:], esm[:], s["m16"][:, :, n0:n0 + NT],
                        start=True, stop=True, perf_mode=DR,
                    )
                    mp = mpt[:]
                for u in range(2):
                    eng = nc.vector
                    eng.scalar_tensor_tensor(
                        out=s["xm"][:, u, n0:n0 + NT],
                        in0=s["x"][:, u, n0:n0 + NT],
                        scalar=0.0,
                        in1=mp,
                        op0=BYPASS,
                        op1=MULT,
                        accum_out=s["ysum"][:, u, j:j + 1],
                    )
                if det_fp8:
                    xe = xq_b0 if (b == 0 and xq_b0) else xq_eng
                    sl = (slice(None), slice(None), slice(n0, n0 + NT))
                    if xe == "pool":
                        nc.gpsimd.tensor_copy(s["xq"][sl], s["xm"][sl])
                    elif xe == "act":
                        nc.scalar.activation(s["xq"][sl], s["xm"][sl], COPY)
                    elif xe == "dve":
                        nc.vector.tensor_copy(s["xq"][sl], s["xm"][sl])
                    else:  # split: ACT u0, DVE u1
                        nc.scalar.activation(
                            s["xq"][:, 0, n0:n0 + NT], s["xm"][:, 0, n0:n0 + NT],
                            COPY)
                        nc.vector.tensor_copy(
                            s["xq"][:, 1, n0:n0 + NT], s["xm"][:, 1, n0:n0 + NT])

            def p1_tail(b):
                s = st[b]
                ysum = s["ysum"]
                if debug:
                    nc.sync.dma_start(dxm_d[b], s["xm"][:].bitcast(f32))
                # SE chain -> per-channel output scale mc[h]
                nc.vector.reduce_sum(ysum[:, :, 7:8], ysum[:, :, 0:NTILES],
                                     axis=mybir.AxisListType.X)
                fc1p = r2ppool.tile([16, 1], f32, name=f"fc1p_b{b}", tag="r2p")
                nc.tensor.matmul(fc1p[:], wfc1[0][:], ysum[:, 0, 7:8],
                                 start=True, stop=False)
                nc.tensor.matmul(fc1p[:], wfc1[1][:], ysum[:, 1, 7:8],
                                 start=False, stop=True)
                h1 = sepool.tile([16, 1], f32, name=f"h1_b{b}", tag="h1")
                nc.scalar.activation(h1[:], fc1p[:], RELU)
                mc = []
                for h in range(2):
                    scp = r2ppool.tile([128, 1], f32, name=f"scp_b{b}h{h}", tag="r2p")
                    nc.tensor.matmul(scp[:], wfc2[h][:], h1[:],
                                     start=True, stop=True)
                    ssb = sepool.tile([128, 1], f32, name=f"ssb_b{b}h{h}", tag="ssb")
                    nc.scalar.activation(ssb[:], scp[:], SIGMOID)
                    mch = mcpool.tile([128, 1], f32, name=f"mc_b{b}h{h}", tag="mc")
                    if mc_dve:
                        # relu(ssb + abias) on DVE keeps ACT off the window path
                        nc.vector.tensor_scalar(
                            out=mch[:], in0=ssb[:], scalar1=abias[h][:],
                            scalar2=0.0,
                            op0=mybir.AluOpType.add, op1=mybir.AluOpType.max)
                    else:
                        nc.scalar.activation(mch[:], ssb[:], RELU,
                                             bias=abias[h][:])
                    mc.append(mch)
                if debug:
                    nc.sync.dma_start(dy_d[b], ysum[:])
                    for h in range(2):
                        nc.sync.dma_start(dmc_d[b, h], mc[h][:])
                s["mc"] = mc

            def det_block(b, j):
                """det GEMM -> sigmoid -> zh, for tile (b, j)."""
                s = st[b]
                n0 = j * NT
                sg = sgpool.tile([128, 2, NT], bf16, name=f"sg_b{b}j{j}", tag="sg")
                dp = (None if sig2 else
                      dppool.tile([128, 2, 512], f32, name=f"dp_b{b}j{j}",
                                  tag="dp"))
                for t in range(2):
                    dpt = (dppool.tile([128, 512], f32, name=f"dp_b{b}j{j}t{t}",
                                       tag="dp")
                           if sig2 else dp[:, t])
                    if det_fp8:
                        nc.tensor.matmul(
                            dpt[:, 0:NT], wf8[:, t], s["xq"][:, :, n0:n0 + NT],
                            start=True, stop=True, perf_mode=DR,
                        )
                    else:
                        for u in range(2):
                            nc.tensor.matmul(
                                dpt[:, 0:NT], wdet_bf[u][t][:],
                                s["xm"][:, u, n0:n0 + NT],
                                start=(u == 0), stop=(u == 1),
                            )
                    if sig2:
                        nc.scalar.activation(sg[:, t, :], dpt[:, 0:NT], SIGMOID)
                if not sig2:
                    nc.scalar.activation(sg[:], dp[:, :, 0:NT], SIGMOID)
                zh = zhpool.tile([128, 2, NT], bf16, name=f"zh_b{b}j{j}", tag="zh")
                nc.vector.tensor_tensor(zh[:], sg[:], s["xm"][:, :, n0:n0 + NT],
                                        MULT)
                s[("zh", j)] = zh

            def rec1_block(b, j):
                s = st[b]
                n0 = j * NT
                zh = s.pop(("zh", j))
                r1p = r1ppool.tile([128, 2, 512], f32, name=f"r1p_b{b}j{j}",
                                   tag="r1p")
                for t in range(2):
                    kts = [(wr1[0][0][t], s["xm"][:, 0, n0:n0 + NT]),
                           (wr1[0][1][t], s["xm"][:, 1, n0:n0 + NT]),
                           (wr1[1][0][t], zh[:, 0, :]),
                           (wr1[1][1][t], zh[:, 1, :])]
                    for k, (wk, mk) in enumerate(kts):
                        nc.tensor.matmul(r1p[:, t, 0:NT], wk[:], mk,
                                         start=(k == 0), stop=(k == 3))
                r1sb = r1pool.tile([128, 2, NT], bf16, name=f"r1_b{b}j{j}", tag="r1")
                nc.scalar.activation(r1sb[:], r1p[:, :, 0:NT], RELU)
                s[("r1", j)] = r1sb

            def rec2_block(b, j):
                s = st[b]
                n0 = j * NT
                r1sb = s.pop(("r1", j))
                mc = s["mc"]
                for h in range(2):
                    r2p = r2ppool.tile([128, NT], f32, name=f"r2p_b{b}h{h}j{j}",
                                       tag="r2p")
                    for u in range(2):
                        nc.tensor.matmul(r2p[:], wr2[u][h][:], r1sb[:, u, :],
                                         start=(u == 0), stop=(u == 1))
                    if j % 4 == 0:
                        ot[(b, h)] = opool.tile([128, 4 * NT], odt,
                                                name=f"o_b{b}h{h}j{j}", tag=f"o{h}")
                    o = ot[(b, h)][:, (j % 4) * NT:(j % 4 + 1) * NT]
                    if h == 0:
                        nc.vector.tensor_scalar_mul(o, r2p[:], mc[0][:])
                    else:
                        nc.scalar.activation(o, r2p[:], COPY, scale=mc[1][:])
                    if j % 4 == 3 or j == NTILES - 1:
                        w = (j % 4 + 1) * NT
                        nc.sync.dma_start(
                            out_d[b, h, :, n0 - (j % 4) * NT:n0 + NT],
                            ot[(b, h)][:, 0:w])

            import contextlib as _ctxlib
            rep_cm = (tc.For_i(0, repeat, 1,
                               hint_engines=(mybir.EngineType.PE,
                                             mybir.EngineType.DVE,
                                             mybir.EngineType.Activation,
                                             mybir.EngineType.SP,
                                             mybir.EngineType.Pool))
                      if repeat else _ctxlib.nullcontext())
            with rep_cm:
                if sched == "pipe":
                    # 2-deep software pipeline over flat (b, j) windows:
                    # window (b,j) emits det(b,j+1), rec1(b,j), rec2(b,j-1)
                    # plus one phase1 tile of image b+1.
                    PRO = NTILES  # image-0 phase1 tiles before windows
                    DRE = 1       # rec2 emission delay in windows
                    for rep in range(unroll):
                        p1_head(0)
                        for j in range(PRO):
                            p1_tile(0, j)
                        if PRO == NTILES:
                            p1_tail(0)
                        det_block(0, 0)
                        units = [(b, j) for b in range(BC)
                                 for j in range(NTILES)]
                        for w, (b, j) in enumerate(units):
                            if w + 1 < len(units):
                                det_block(*units[w + 1])
                            rec1_block(b, j)
                            if w >= DRE:
                                rec2_block(*units[w - DRE])
                            if b == 0 and j + PRO < NTILES:
                                p1_tile(0, j + PRO)
                                if j + PRO == NTILES - 1:
                                    p1_tail(0)
                            if b + 1 < BC:
                                if j == 0:
                                    p1_head(b + 1)
                                p1_tile(b + 1, j)
                                if j == NTILES - 1:
                                    p1_tail(b + 1)
                        for w in range(len(units) - DRE, len(units)):
                            rec2_block(*units[w])
                        st.clear()
                else:
                    for b in range(BC):
                        p1_head(b)
                        for j in range(NTILES):
                            p1_tile(b, j)
                        p1_tail(b)
                        for j in range(NTILES):
                            det_block(b, j)
                            rec1_block(b, j)
                            rec2_block(b, j)
                        del st[b]

    _split_multiwaits(nc, mybir)
    return nc


def _jax_perm_cpu(num_chunks: int) -> np.ndarray:
    """jax.random.permutation(key(1234), num_chunks) on the CPU backend.

    Run in a JAX_PLATFORMS=cpu subprocess: in this process jax may be bound
    to an accelerator backend that cannot lower the shuffle's sort op.
    """
    import os
    import subprocess
    import sys
    import tempfile

    import jax

    sp = os.path.dirname(os.path.dirname(jax.__file__))
    code = (
        "import sys, numpy as np, jax\n"
        f"perm = np.asarray(jax.random.permutation(jax.random.key(1234), {num_chunks}))\n"
        "np.save(sys.argv[1], perm)\n"
    )
    with tempfile.TemporaryDirectory() as td:
        path = os.path.join(td, "perm.npy")
        env = dict(os.environ, JAX_PLATFORMS="cpu", PYTHONPATH=sp)
        env.pop("TRN_TERMINAL_POOL_IPS", None)
        subprocess.run([sys.executable, "-c", code, path], env=env, check=True)
        return np.load(path)


def _mask16(rate: int) -> np.ndarray:
    """Per-image [16, HW] fp8 keep-mask in channel-group space."""
    n = B * C * HW
    num_chunks = math.ceil(n * 4 / 1472)
    num_lossy = int(math.ceil(num_chunks * (rate / 100)))
    perm = _jax_perm_cpu(num_chunks)
    keep = np.ones((num_chunks,), np.float32)
    if num_lossy > 0:
        keep[perm[:num_lossy]] = 0.0
    bg = np.arange(B, dtype=np.int64)
    qq = np.arange(QG, dtype=np.int64)
    pp = np.arange(HW, dtype=np.int64)
    u = (bg[:, None, None] * HW + pp[None, None, :]) * QG + qq[None, :, None]
    return keep[u // UPC].astype(ml_dtypes.float8_e4m3)


def _prep_in_maps(inputs, det_fp8=False, mask_dma=True):
    x = np.asarray(inputs["x"], dtype=np.float32)
    rate = int(np.asarray(inputs["Packet_Loss_Rate"]))
    fc1 = np.asarray(inputs["fc1_w"], dtype=np.float32)
    fc2 = np.asarray(inputs["fc2_w"], dtype=np.float32)
    thr = float(np.asarray(inputs["threshold"], dtype=np.float32).reshape(-1)[0])
    detw = np.asarray(inputs["detect_w"], dtype=np.float32)
    r1w = np.asarray(inputs["rec1_w"], dtype=np.float32)
    r2w = np.asarray(inputs["rec2_w"], dtype=np.float32)
    aw = np.asarray(inputs["adapt_w"], dtype=np.float32)

    bf = ml_dtypes.bfloat16
    f8 = ml_dtypes.float8_e4m3

    # x: [B, C, HW] -> [B, 128, 2, HW] bf16 (c = 2p + u)
    xr = np.ascontiguousarray(
        x.reshape(B, 128, 2, HW).astype(bf))

    # bf16 weight blob [128, ncol, 128]
    detT = detw.T.reshape(128, 2, 128, 2)               # [p, u, q, t]
    r1T = r1w.T.reshape(2, 128, 2, 128, 2)              # [kt, p, u, q, t]
    r2T = r2w.T.reshape(128, 2, 2, 128)                 # [p, u, h, q]
    blocks = []
    if not det_fp8:
        blocks.append(detT.transpose(0, 1, 3, 2).reshape(128, 4, 128))
    blocks.append(r1T.transpose(1, 0, 2, 4, 3).reshape(128, 8, 128))
    blocks.append(r2T.reshape(128, 4, 128))
    wbf = np.ascontiguousarray(np.concatenate(blocks, axis=1).astype(bf))

    # E2 expansion: [8, 2, 128], 1 iff group(2q) == 2*pg + ug
    pg = np.arange(8)
    ug = np.arange(2)
    q = np.arange(128)
    esm = ((2 * pg[:, None, None] + ug[None, :, None]) ==
           (q[None, None, :] // 8)).astype(f8)

    wfc = np.zeros((128, 34), np.float32)
    fc1T = (fc1.T / HW).reshape(128, 2, 16)             # [p, u, m]
    wfc[:, 0:16] = fc1T[:, 0, :]
    wfc[:, 16:32] = fc1T[:, 1, :]
    ab = (rate * aw[:, 0] - thr).astype(np.float32)
    wfc[:, 32] = ab[0:128]
    wfc[:, 33] = ab[128:256]

    wsm = np.ascontiguousarray(fc2.T.astype(np.float32))  # [16, 256]

    if mask_dma:
        m16 = np.ascontiguousarray(_mask16(rate).repeat(8, axis=1))
    else:
        m16 = _mask16(rate).reshape(B, 8, 2, HW)

    in_maps = []
    for c in range(NCORES):
        m = {
            "x": xr[c * BC:(c + 1) * BC],
            "m16": m16[c * BC:(c + 1) * BC],
            "wbf": wbf, "esm": esm, "wfc": wfc, "wsm": wsm,
        }
        if det_fp8:
            m["wf8"] = np.ascontiguousarray(
                detT.transpose(0, 3, 1, 2).astype(f8))  # [p, t, u, q]
        if mask_dma:
            del m["esm"]
        in_maps.append(m)
    return in_maps


BUILD_KW = dict(det_fp8=False, xq_eng="pool", sched="pipe", out_bf16=True,
                sig2=True, mc_dve=True, mask_dma=True)


def kernel(**inputs) -> np.ndarray:
    from concourse.bass_utils import run_bass_kernel_spmd

    kw = _CACHE.get("kw", BUILD_KW)
    in_maps = _prep_in_maps(inputs, det_fp8=kw.get("det_fp8", False),
                            mask_dma=kw.get("mask_dma", False))
    if "nc" not in _CACHE:
        _CACHE["nc"] = _build(**kw)
    nc = _CACHE["nc"]
    last_err = None
    for _attempt in range(3):
        try:
            res = run_bass_kernel_spmd(nc, in_maps, core_ids=list(range(NCORES)))
            break
        except Exception as e:  # transient axon/device hiccups: retry
            last_err = e
    else:
        raise last_err
    out = np.stack([res.results[c]["out"] for c in range(NCORES)], axis=0)
    return out.reshape(B, C, H, W).astype(np.float32)


# revision 5
# speedup vs baseline: 1.1584x; 1.0988x over previous
"""EnhancedChannelFilter Trainium2 kernel (pair-interleaved layout).

Full inputs in, full outputs out. Pure data-parallel over 8 NeuronCores
(4 images each). Channels are pair-interleaved on SBUF: channel c = 2p+u
lives at partition p, sub-row u, so tiles are [128, 2, pixels].

Per core, per image:
  1. x shipped bf16 [128, 2, HW]; packet-loss keep-mask shipped fp8
     PRE-EXPANDED to [128, HW] (exact 0/1 values; trading ~1.4 MB/core of
     spare DMA bandwidth for 28 PE mask-expand matmuls + their DoubleRow
     LDWEIGHTS reloads and a PSUM round-trip -- measured ~9 us faster on HW).
  2. xm = x * mask fused with the SE row-sum via DVE STT accum_out (the
     f32 accumulation happens pre-bf16-rounding, keeping the SE path exact).
  3. SE chain (fc1 -> relu -> fc2 -> sigmoid -> +bias -> relu) on PE/ACT/DVE;
     1/HW and rate*adapt_w - threshold folded into host-packed weights.
  4. det/rec1/rec2 GEMMs all bf16 (fp8 DoubleRow for det measured SLOWER on
     hardware despite the cost model -- LDWEIGHTS pays +72% in DR mode);
     sigmoid/relu PSUM evictions on ACT (and nothing else -- ACT's real
     per-op cost is above the model, so both final per-channel scales run
     on DVE: measured ~8 us faster), zh = sigmoid(det)*xm on DVE, bf16 out
     tiles, 4-tile-grouped out DMAs (host upcasts to f32).

Scheduling: a 2-deep software pipeline over flat (image, tile) windows --
window w emits det(w+1), rec1(w), rec2(w-1) plus one phase-1 tile of the
next image -- so the PE never waits on the det->sigmoid->zh chain. DMA
issue is kept off the critical path (each dma_start costs ~650 ns of
sequencer + HWDGE time, serially per queue): one x DMA per sub-row, one
mask DMA per image, constants issued from the ACT queue.
"""

import math

import numpy as np
import ml_dtypes

B, C, H, W = 32, 256, 56, 56
HW = H * W              # 3136
NCORES = 8
BC = B // NCORES        # images per core
NT = 448                # pixels per n-tile
NTILES = HW // NT       # 7
EPC = 1472 // 4         # f32 elements per packet chunk (368)
QG = 16                 # channel-group size: gcd(EPC, C)
UPC = EPC // QG         # 23 channel-group-units per chunk

_CACHE: dict = {}


# ---------------------------------------------------------------------------
# Workaround: this walrus build enforces 1 sync wait per instruction (2 for
# EventSemaphore), but the Tile framework attaches several to its exit drain.
# Splitting extra waits onto dedicated same-engine NOPs placed immediately
# before the instruction is semantically identical.
# ---------------------------------------------------------------------------
def _split_multiwaits(nc, mybir):
    n = 0
    for bb in nc.m.functions[0].blocks:
        lst = bb.instructions
        for inst in list(lst):
            si = inst.sync_info
            if si is None or not si.on_wait:
                continue
            cap = 2 if isinstance(inst, mybir.InstEventSemaphore) else 1
            waits = list(si.on_wait)
            if len(waits) <= cap:
                continue
            eng = nc.engines[inst.engine]
            extra = []
            for wt in waits[:-cap]:
                nop = eng.nop(nofuse=True).ins
                nop.sync_info = mybir.SyncInfo(on_wait=[wt], on_update=[])
                nc.cur_bb.bb.instructions.remove(nop)
                extra.append(nop)
            si.on_wait = waits[-cap:]
            idx = lst.index(inst)
            lst[idx:idx] = extra
            n += 1
    return n


def _build(debug=False, repeat=0, det_fp8=False, xq_eng="pool", sched="pipe",
           out_bf16=True, sig2=True, mc_dve=True, xq_b0=None, unroll=1,
           mask_dma=True, fin_dve=True):
    import concourse.bass as bass
    import concourse.tile as tile
    import concourse.mybir as mybir

    f32 = mybir.dt.float32
    bf16 = mybir.dt.bfloat16
    fp8 = mybir.dt.float8e4
    DR = mybir.MatmulPerfMode.DoubleRow
    MULT = mybir.AluOpType.mult
    BYPASS = mybir.AluOpType.bypass
    SIGMOID = mybir.ActivationFunctionType.Sigmoid
    COPY = mybir.ActivationFunctionType.Copy
    RELU = mybir.ActivationFunctionType.Relu

    dp_bufs = (2 if mask_dma else 1) if not sig2 else 2
    nc = bass.Bass("TRN2", target_bir_lowering=False, debug=False)

    x_d = nc.dram_tensor("x", [BC, 128, 2, HW], bf16, kind="ExternalInput").ap()
    if mask_dma:
        m16_d = nc.dram_tensor("m16", [BC, 128, HW], fp8,
                               kind="ExternalInput").ap()
    else:
        m16_d = nc.dram_tensor("m16", [BC, 8, 2, HW], fp8,
                               kind="ExternalInput").ap()
    # bf16 GEMM weights: [128, ncol, 128] with col blocks
    #   det (u,t) 4 cols (absent when det_fp8), rec1 (kt,u,t) 8, rec2 (u,h) 4
    nbcol = (0 if det_fp8 else 4) + 8 + 4
    wbf_d = nc.dram_tensor("wbf", [128, nbcol, 128], bf16, kind="ExternalInput").ap()
    if det_fp8:
        wf8_d = nc.dram_tensor("wf8", [128, 2, 2, 128], fp8, kind="ExternalInput").ap()
    if not mask_dma:
        esm_d = nc.dram_tensor("esm", [8, 2, 128], fp8,
                               kind="ExternalInput").ap()
    # f32 smalls: fc1 (u) 2x16 cols then abias (h) 2x1
    wfc_d = nc.dram_tensor("wfc", [128, 34], f32, kind="ExternalInput").ap()
    wsm_d = nc.dram_tensor("wsm", [16, 256], f32, kind="ExternalInput").ap()
    odt = bf16 if out_bf16 else f32
    out_d = nc.dram_tensor("out", [BC, 2, 128, HW], odt, kind="ExternalOutput").ap()
    if debug:
        dxm_d = nc.dram_tensor("dxm", [BC, 128, 2, HW], f32, kind="ExternalOutput").ap()
        dsg_d = nc.dram_tensor("dsg", [BC, 128, 2, HW], f32, kind="ExternalOutput").ap()
        dmc_d = nc.dram_tensor("dmc", [BC, 2, 128, 1], f32, kind="ExternalOutput").ap()
        dy_d = nc.dram_tensor("dy", [BC, 128, 2, 8], f32, kind="ExternalOutput").ap()

    with tile.TileContext(nc) as tc:
        with (
            tc.tile_pool(name="consts", bufs=1) as cpool,
            tc.tile_pool(name="xin", bufs=3) as xpool,
            tc.tile_pool(name="xm", bufs=2) as xmpool,
            tc.tile_pool(name="xq", bufs=2) as xqpool,
            tc.tile_pool(name="m16", bufs=3) as m16pool,
            tc.tile_pool(name="sg", bufs=3) as sgpool,
            tc.tile_pool(name="zh", bufs=3) as zhpool,
            tc.tile_pool(name="r1", bufs=6) as r1pool,
            tc.tile_pool(name="osb", bufs=3) as opool,
            tc.tile_pool(name="ysum", bufs=2) as ypool,
            tc.tile_pool(name="mch", bufs=4) as mcpool,
            tc.tile_pool(name="sesb", bufs=2) as sepool,
            tc.tile_pool(name="mp", bufs=2, space="PSUM") as mppool,
            tc.tile_pool(name="dp", bufs=dp_bufs, space="PSUM") as dppool,
            tc.tile_pool(name="r1p", bufs=1, space="PSUM") as r1ppool,
            tc.tile_pool(name="r2p", bufs=2, space="PSUM") as r2ppool,
        ):
            # ---- constants into SBUF ----
            wbf = cpool.tile([128, nbcol, 128], bf16, name="wbf", tag="wbf")
            wfc = cpool.tile([128, 34], f32, name="wfc", tag="wfc")
            wsm = cpool.tile([16, 256], f32, name="wsm", tag="wsm")
            if not mask_dma:
                esm = cpool.tile([8, 2, 128], fp8, name="esm", tag="esm")
                nc.scalar.dma_start(esm[:], esm_d[:])
            nc.scalar.dma_start(wfc[:], wfc_d[:])
            nc.scalar.dma_start(wsm[:], wsm_d[:])
            if det_fp8:
                wf8 = cpool.tile([128, 2, 2, 128], fp8, name="wf8", tag="wf8")
                nc.sync.dma_start(wf8[:], wf8_d[:])
            if repeat:
                nc.sync.dma_start(wbf[:], wbf_d[:])

            co = 0 if det_fp8 else 4
            if not det_fp8:
                wdet_bf = [[wbf[:, u * 2 + t] for t in range(2)] for u in range(2)]
            # rec1 col block: (kt, u, t)
            wr1 = [[[wbf[:, co + (kt * 2 + u) * 2 + t] for t in range(2)]
                    for u in range(2)] for kt in range(2)]
            wr2 = [[wbf[:, co + 8 + u * 2 + h] for h in range(2)] for u in range(2)]
            wfc1 = [wfc[:, u * 16:(u + 1) * 16] for u in range(2)]
            abias = [wfc[:, 32 + h:33 + h] for h in range(2)]
            wfc2 = [wsm[:, h * 128:(h + 1) * 128] for h in range(2)]

            st = {}      # b -> (xm_sb, xq_sb, mc)
            ot = {}      # (b, h) -> out SBUF tile

            def p1_head(b):
                if mask_dma:
                    m16_sb = m16pool.tile([128, HW], fp8, name=f"m16_b{b}",
                                          tag="m16")
                    nc.sync.dma_start(m16_sb[:], m16_d[b])
                else:
                    m16_sb = m16pool.tile([8, 2, HW], fp8, name=f"m16_b{b}",
                                          tag="m16")
                    nc.sync.dma_start(m16_sb[:], m16_d[b])
                x_sb = xpool.tile([128, 2, HW], bf16, name=f"x_b{b}", tag="x")
                if b == 0 and not repeat:
                    nc.sync.dma_start(wbf[:], wbf_d[:])
                # one DMA per sub-row: fewer 650ns issue slots on the SP queue
                for u in range(2):
                    nc.sync.dma_start(x_sb[:, u, :], x_d[b, :, u, :])

                xm_sb = xmpool.tile([128, 2, HW], bf16, name=f"xm_b{b}", tag="xm")
                xq_sb = (xqpool.tile([128, 2, HW], fp8, name=f"xq_b{b}", tag="xq")
                         if det_fp8 else None)
                ysum = ypool.tile([128, 2, 8], f32, name=f"ysum_b{b}", tag="ysum")
                st[b] = dict(m16=m16_sb, x=x_sb, xm=xm_sb, xq=xq_sb, ysum=ysum)

            def p1_tile(b, j):
                s = st[b]
                n0 = j * NT
                if mask_dma:
                    mp = s["m16"][:, n0:n0 + NT]
                else:
                    mpt = mppool.tile([128, NT], f32, name=f"mp_b{b}j{j}",
                                      tag="mp")
                    nc.tensor.matmul(
                        mpt[:], esm[:], s["m16"][:, :, n0:n0 + NT],
                        start=True, stop=True, perf_mode=DR,
                    )
                    mp = mpt[:]
                for u in range(2):
                    eng = nc.vector
                    eng.scalar_tensor_tensor(
                        out=s["xm"][:, u, n0:n0 + NT],
                        in0=s["x"][:, u, n0:n0 + NT],
                        scalar=0.0,
                        in1=mp,
                        op0=BYPASS,
                        op1=MULT,
                        accum_out=s["ysum"][:, u, j:j + 1],
                    )
                if det_fp8:
                    xe = xq_b0 if (b == 0 and xq_b0) else xq_eng
                    sl = (slice(None), slice(None), slice(n0, n0 + NT))
                    if xe == "pool":
                        nc.gpsimd.tensor_copy(s["xq"][sl], s["xm"][sl])
                    elif xe == "act":
                        nc.scalar.activation(s["xq"][sl], s["xm"][sl], COPY)
                    elif xe == "dve":
                        nc.vector.tensor_copy(s["xq"][sl], s["xm"][sl])
                    else:  # split: ACT u0, DVE u1
                        nc.scalar.activation(
                            s["xq"][:, 0, n0:n0 + NT], s["xm"][:, 0, n0:n0 + NT],
                            COPY)
                        nc.vector.tensor_copy(
                            s["xq"][:, 1, n0:n0 + NT], s["xm"][:, 1, n0:n0 + NT])

            def p1_tail(b):
                s = st[b]
                ysum = s["ysum"]
                if debug:
                    nc.sync.dma_start(dxm_d[b], s["xm"][:].bitcast(f32))
                # SE chain -> per-channel output scale mc[h]
                nc.vector.reduce_sum(ysum[:, :, 7:8], ysum[:, :, 0:NTILES],
                                     axis=mybir.AxisListType.X)
                fc1p = r2ppool.tile([16, 1], f32, name=f"fc1p_b{b}", tag="r2p")
                nc.tensor.matmul(fc1p[:], wfc1[0][:], ysum[:, 0, 7:8],
                                 start=True, stop=False)
                nc.tensor.matmul(fc1p[:], wfc1[1][:], ysum[:, 1, 7:8],
                                 start=False, stop=True)
                h1 = sepool.tile([16, 1], f32, name=f"h1_b{b}", tag="h1")
                nc.scalar.activation(h1[:], fc1p[:], RELU)
                mc = []
                for h in range(2):
                    scp = r2ppool.tile([128, 1], f32, name=f"scp_b{b}h{h}", tag="r2p")
                    nc.tensor.matmul(scp[:], wfc2[h][:], h1[:],
                                     start=True, stop=True)
                    ssb = sepool.tile([128, 1], f32, name=f"ssb_b{b}h{h}", tag="ssb")
                    nc.scalar.activation(ssb[:], scp[:], SIGMOID)
                    mch = mcpool.tile([128, 1], f32, name=f"mc_b{b}h{h}", tag="mc")
                    if mc_dve:
                        # relu(ssb + abias) on DVE keeps ACT off the window path
                        nc.vector.tensor_scalar(
                            out=mch[:], in0=ssb[:], scalar1=abias[h][:],
                            scalar2=0.0,
                            op0=mybir.AluOpType.add, op1=mybir.AluOpType.max)
                    else:
                        nc.scalar.activation(mch[:], ssb[:], RELU,
                                             bias=abias[h][:])
                    mc.append(mch)
                if debug:
                    nc.sync.dma_start(dy_d[b], ysum[:])
                    for h in range(2):
                        nc.sync.dma_start(dmc_d[b, h], mc[h][:])
                s["mc"] = mc

            def det_block(b, j):
                """det GEMM -> sigmoid -> zh, for tile (b, j)."""
                s = st[b]
                n0 = j * NT
                sg = sgpool.tile([128, 2, NT], bf16, name=f"sg_b{b}j{j}", tag="sg")
                dp = (None if sig2 else
                      dppool.tile([128, 2, 512], f32, name=f"dp_b{b}j{j}",
                                  tag="dp"))
                for t in range(2):
                    dpt = (dppool.tile([128, 512], f32, name=f"dp_b{b}j{j}t{t}",
                                       tag="dp")
                           if sig2 else dp[:, t])
                    if det_fp8:
                        nc.tensor.matmul(
                            dpt[:, 0:NT], wf8[:, t], s["xq"][:, :, n0:n0 + NT],
                            start=True, stop=True, perf_mode=DR,
                        )
                    else:
                        for u in range(2):
                            nc.tensor.matmul(
                                dpt[:, 0:NT], wdet_bf[u][t][:],
                                s["xm"][:, u, n0:n0 + NT],
                                start=(u == 0), stop=(u == 1),
                            )
                    if sig2:
                        nc.scalar.activation(sg[:, t, :], dpt[:, 0:NT], SIGMOID)
                if not sig2:
                    nc.scalar.activation(sg[:], dp[:, :, 0:NT], SIGMOID)
                zh = zhpool.tile([128, 2, NT], bf16, name=f"zh_b{b}j{j}", tag="zh")
                nc.vector.tensor_tensor(zh[:], sg[:], s["xm"][:, :, n0:n0 + NT],
                                        MULT)
                s[("zh", j)] = zh

            def rec1_block(b, j):
                s = st[b]
                n0 = j * NT
                zh = s.pop(("zh", j))
                r1p = r1ppool.tile([128, 2, 512], f32, name=f"r1p_b{b}j{j}",
                                   tag="r1p")
                for t in range(2):
                    kts = [(wr1[0][0][t], s["xm"][:, 0, n0:n0 + NT]),
                           (wr1[0][1][t], s["xm"][:, 1, n0:n0 + NT]),
                           (wr1[1][0][t], zh[:, 0, :]),
                           (wr1[1][1][t], zh[:, 1, :])]
                    for k, (wk, mk) in enumerate(kts):
                        nc.tensor.matmul(r1p[:, t, 0:NT], wk[:], mk,
                                         start=(k == 0), stop=(k == 3))
                r1sb = r1pool.tile([128, 2, NT], bf16, name=f"r1_b{b}j{j}", tag="r1")
                nc.scalar.activation(r1sb[:], r1p[:, :, 0:NT], RELU)
                s[("r1", j)] = r1sb

            def rec2_block(b, j):
                s = st[b]
                n0 = j * NT
                r1sb = s.pop(("r1", j))
                mc = s["mc"]
                for h in range(2):
                    r2p = r2ppool.tile([128, NT], f32, name=f"r2p_b{b}h{h}j{j}",
                                       tag="r2p")
                    for u in range(2):
                        nc.tensor.matmul(r2p[:], wr2[u][h][:], r1sb[:, u, :],
                                         start=(u == 0), stop=(u == 1))
                    if j % 4 == 0:
                        ot[(b, h)] = opool.tile([128, 4 * NT], odt,
                                                name=f"o_b{b}h{h}j{j}", tag=f"o{h}")
                    o = ot[(b, h)][:, (j % 4) * NT:(j % 4 + 1) * NT]
                    if h == 0 or fin_dve:
                        nc.vector.tensor_scalar_mul(o, r2p[:], mc[h][:])
                    else:
                        nc.scalar.activation(o, r2p[:], COPY, scale=mc[1][:])
                    if j % 4 == 3 or j == NTILES - 1:
                        w = (j % 4 + 1) * NT
                        nc.sync.dma_start(
                            out_d[b, h, :, n0 - (j % 4) * NT:n0 + NT],
                            ot[(b, h)][:, 0:w])

            import contextlib as _ctxlib
            rep_cm = (tc.For_i(0, repeat, 1,
                               hint_engines=(mybir.EngineType.PE,
                                             mybir.EngineType.DVE,
                                             mybir.EngineType.Activation,
                                             mybir.EngineType.SP,
                                             mybir.EngineType.Pool))
                      if repeat else _ctxlib.nullcontext())
            with rep_cm:
                if sched == "pipe":
                    # 2-deep software pipeline over flat (b, j) windows:
                    # window (b,j) emits det(b,j+1), rec1(b,j), rec2(b,j-1)
                    # plus one phase1 tile of image b+1.
                    PRO = NTILES  # image-0 phase1 tiles before windows
                    DRE = 1       # rec2 emission delay in windows
                    for rep in range(unroll):
                        p1_head(0)
                        for j in range(PRO):
                            p1_tile(0, j)
                        if PRO == NTILES:
                            p1_tail(0)
                        det_block(0, 0)
                        units = [(b, j) for b in range(BC)
                                 for j in range(NTILES)]
                        for w, (b, j) in enumerate(units):
                            if w + 1 < len(units):
                                det_block(*units[w + 1])
                            rec1_block(b, j)
                            if w >= DRE:
                                rec2_block(*units[w - DRE])
                            if b == 0 and j + PRO < NTILES:
                                p1_tile(0, j + PRO)
                                if j + PRO == NTILES - 1:
                                    p1_tail(0)
                            if b + 1 < BC:
                                if j == 0:
                                    p1_head(b + 1)
                                p1_tile(b + 1, j)
                                if j == NTILES - 1:
                                    p1_tail(b + 1)
                        for w in range(len(units) - DRE, len(units)):
                            rec2_block(*units[w])
                        st.clear()
                else:
                    for b in range(BC):
                        p1_head(b)
                        for j in range(NTILES):
                            p1_tile(b, j)
                        p1_tail(b)
                        for j in range(NTILES):
                            det_block(b, j)
                            rec1_block(b, j)
                            rec2_block(b, j)
                        del st[b]

    _split_multiwaits(nc, mybir)
    return nc


def _jax_perm_cpu(num_chunks: int) -> np.ndarray:
    """jax.random.permutation(key(1234), num_chunks) on the CPU backend.

    Run in a JAX_PLATFORMS=cpu subprocess: in this process jax may be bound
    to an accelerator backend that cannot lower the shuffle's sort op.
    """
    import os
    import subprocess
    import sys
    import tempfile

    import jax

    sp = os.path.dirname(os.path.dirname(jax.__file__))
    code = (
        "import sys, numpy as np, jax\n"
        f"perm = np.asarray(jax.random.permutation(jax.random.key(1234), {num_chunks}))\n"
        "np.save(sys.argv[1], perm)\n"
    )
    with tempfile.TemporaryDirectory() as td:
        path = os.path.join(td, "perm.npy")
        env = dict(os.environ, JAX_PLATFORMS="cpu", PYTHONPATH=sp)
        env.pop("TRN_TERMINAL_POOL_IPS", None)
        subprocess.run([sys.executable, "-c", code, path], env=env, check=True)
        return np.load(path)


def _mask16(rate: int) -> np.ndarray:
    """Per-image [16, HW] fp8 keep-mask in channel-group space."""
    n = B * C * HW
    num_chunks = math.ceil(n * 4 / 1472)
    num_lossy = int(math.ceil(num_chunks * (rate / 100)))
    perm = _jax_perm_cpu(num_chunks)
    keep = np.ones((num_chunks,), np.float32)
    if num_lossy > 0:
        keep[perm[:num_lossy]] = 0.0
    bg = np.arange(B, dtype=np.int64)
    qq = np.arange(QG, dtype=np.int64)
    pp = np.arange(HW, dtype=np.int64)
    u = (bg[:, None, None] * HW + pp[None, None, :]) * QG + qq[None, :, None]
    return keep[u // UPC].astype(ml_dtypes.float8_e4m3)


def _prep_in_maps(inputs, det_fp8=False, mask_dma=True):
    x = np.asarray(inputs["x"], dtype=np.float32)
    rate = int(np.asarray(inputs["Packet_Loss_Rate"]))
    fc1 = np.asarray(inputs["fc1_w"], dtype=np.float32)
    fc2 = np.asarray(inputs["fc2_w"], dtype=np.float32)
    thr = float(np.asarray(inputs["threshold"], dtype=np.float32).reshape(-1)[0])
    detw = np.asarray(inputs["detect_w"], dtype=np.float32)
    r1w = np.asarray(inputs["rec1_w"], dtype=np.float32)
    r2w = np.asarray(inputs["rec2_w"], dtype=np.float32)
    aw = np.asarray(inputs["adapt_w"], dtype=np.float32)

    bf = ml_dtypes.bfloat16
    f8 = ml_dtypes.float8_e4m3

    # x: [B, C, HW] -> [B, 128, 2, HW] bf16 (c = 2p + u)
    xr = np.ascontiguousarray(
        x.reshape(B, 128, 2, HW).astype(bf))

    # bf16 weight blob [128, ncol, 128]
    detT = detw.T.reshape(128, 2, 128, 2)               # [p, u, q, t]
    r1T = r1w.T.reshape(2, 128, 2, 128, 2)              # [kt, p, u, q, t]
    r2T = r2w.T.reshape(128, 2, 2, 128)                 # [p, u, h, q]
    blocks = []
    if not det_fp8:
        blocks.append(detT.transpose(0, 1, 3, 2).reshape(128, 4, 128))
    blocks.append(r1T.transpose(1, 0, 2, 4, 3).reshape(128, 8, 128))
    blocks.append(r2T.reshape(128, 4, 128))
    wbf = np.ascontiguousarray(np.concatenate(blocks, axis=1).astype(bf))

    # E2 expansion: [8, 2, 128], 1 iff group(2q) == 2*pg + ug
    pg = np.arange(8)
    ug = np.arange(2)
    q = np.arange(128)
    esm = ((2 * pg[:, None, None] + ug[None, :, None]) ==
           (q[None, None, :] // 8)).astype(f8)

    wfc = np.zeros((128, 34), np.float32)
    fc1T = (fc1.T / HW).reshape(128, 2, 16)             # [p, u, m]
    wfc[:, 0:16] = fc1T[:, 0, :]
    wfc[:, 16:32] = fc1T[:, 1, :]
    ab = (rate * aw[:, 0] - thr).astype(np.float32)
    wfc[:, 32] = ab[0:128]
    wfc[:, 33] = ab[128:256]

    wsm = np.ascontiguousarray(fc2.T.astype(np.float32))  # [16, 256]

    if mask_dma:
        m16 = np.ascontiguousarray(_mask16(rate).repeat(8, axis=1))
    else:
        m16 = _mask16(rate).reshape(B, 8, 2, HW)

    in_maps = []
    for c in range(NCORES):
        m = {
            "x": xr[c * BC:(c + 1) * BC],
            "m16": m16[c * BC:(c + 1) * BC],
            "wbf": wbf, "esm": esm, "wfc": wfc, "wsm": wsm,
        }
        if det_fp8:
            m["wf8"] = np.ascontiguousarray(
                detT.transpose(0, 3, 1, 2).astype(f8))  # [p, t, u, q]
        if mask_dma:
            del m["esm"]
        in_maps.append(m)
    return in_maps


BUILD_KW = dict(det_fp8=False, xq_eng="pool", sched="pipe", out_bf16=True,
                sig2=True, mc_dve=True, mask_dma=True, fin_dve=True)


def kernel(**inputs) -> np.ndarray:
    from concourse.bass_utils import run_bass_kernel_spmd

    kw = _CACHE.get("kw", BUILD_KW)
    in_maps = _prep_in_maps(inputs, det_fp8=kw.get("det_fp8", False),
                            mask_dma=kw.get("mask_dma", False))
    if "nc" not in _CACHE:
        _CACHE["nc"] = _build(**kw)
    nc = _CACHE["nc"]
    last_err = None
    for _attempt in range(3):
        try:
            res = run_bass_kernel_spmd(nc, in_maps, core_ids=list(range(NCORES)))
            break
        except Exception as e:  # transient axon/device hiccups: retry
            last_err = e
    else:
        raise last_err
    out = np.stack([res.results[c]["out"] for c in range(NCORES)], axis=0)
    return out.reshape(B, C, H, W).astype(np.float32)
